# revision 1
# baseline (speedup 1.0000x reference)
"""Trainium2 Bass kernel for nn_ControlledConvEMAStabilizer.

Pipeline (per batch image, one NeuronCore each, batch-parallel over 8 cores):
  q = cat(backbone, z, mem_stab, mem_unstab)          # 160ch
  q = lrelu(conv3x3(q, w0) + b0)                      # -> 64ch
  q = lrelu(conv3x3(q, w1) + b1)                      # -> 64ch
  q = lrelu(conv3x3(q, w2) + b2)                      # -> 64ch
  head = conv3x3(q, w_last) + b_last                  # -> 288ch = 9 taps x 32ch
  eta  = softmax([head; 0]) over the 9+1 slots
  out  = sum_p unfold(mem_stab)[p] * eta[p] + eta[9] * z

Implementation notes:
  - Feature maps live in SBUF as zero-padded flat rows: image pixel (r,c) at
    column 129*(r+1)+1+c  (row stride 129, shared single pad column between
    rows, one pad row top/bottom).  Every 3x3 tap is then a pure column
    offset t = 129*dr + dc, so convs are PSUM-accumulated matmuls over
    shifted views (float32r -> full PE rate at N>=256).
  - K-stacking: each intermediate tensor is stored twice in one [128, NCOL]
    tile: partitions 0:64 = q, partitions 64:128 = q shifted by +129 (one
    image row).  A K=128 matmul then applies two vertical taps at once.
  - LeakyReLU: y = (x + b) + Relu(-0.99*(x + b)), via one ScalarE activation
    (scale=-0.99, bias=-0.99b) + one fused DVE scalar_tensor_tensor.
  - Tail fused per 3-row strip: conv_last (18 mm) -> Exp(+b_last) on ACT ->
    multiply with shifted mem_stab patches (DVE) -> partition-group sums via
    block-identity matmuls (PE) -> reciprocal_approx_fast -> out.
"""

import numpy as np
from contextlib import ExitStack

import concourse.bacc as bacc
import concourse.tile as tile
from concourse import mybir
from concourse.bass_utils import run_bass_kernel_spmd

F32 = mybir.dt.float32
F32R = mybir.dt.float32r
BF16 = mybir.dt.bfloat16
ALU = mybir.AluOpType
ACTF = mybir.ActivationFunctionType

H = 128
ST = 129                      # padded row stride
NCOL = ST * 130 + 2           # 16772 sbuf cols (incl 1 extra tail zero)
XCOL = NCOL                   # dram padded cols for xpad
MUCOL = NCOL + 2 * ST + 2     # mu needs reads up to +258 further
ROWS_PER_STRIP = 3
X_GROUP_STRIPS = 3            # conv0 input staging granularity (9 rows)

# taps in fusion/unfold order p = 3*kh + kw -> offset 129*(kh-1) + (kw-1)
P_TAPS = [ST * (kh - 1) + (kw - 1) for kh in range(3) for kw in range(3)]


def _j0(r0):
    return ST * (r0 + 1) + 1


def _strips():
    out = []
    r0 = 0
    while r0 < H:
        nr = min(ROWS_PER_STRIP, H - r0)
        out.append((r0, nr))
        r0 += nr
    return out


def _build_program(debug=False):
    nc = bacc.Bacc("TRN2", target_bir_lowering=False, debug=False)

    d_xpad = nc.dram_tensor("xpad", [128, XCOL], BF16, kind="ExternalInput")
    d_mupad = nc.dram_tensor("mupad", [32, MUCOL], BF16, kind="ExternalInput")
    d_w0c1 = nc.dram_tensor("w0c1", [128, 9 * 64], BF16, kind="ExternalInput")
    d_w0c2 = nc.dram_tensor("w0c2", [96, 3 * 64], BF16, kind="ExternalInput")
    d_w1P = nc.dram_tensor("w1P", [128, 3 * 64], BF16, kind="ExternalInput")
    d_w1S = nc.dram_tensor("w1S", [64, 3 * 64], BF16, kind="ExternalInput")
    d_w2P = nc.dram_tensor("w2P", [128, 3 * 64], BF16, kind="ExternalInput")
    d_w2S = nc.dram_tensor("w2S", [64, 3 * 64], BF16, kind="ExternalInput")
    d_wlP = nc.dram_tensor("wlP", [128, 3 * 288], BF16, kind="ExternalInput")
    d_wlS = nc.dram_tensor("wlS", [64, 3 * 288], BF16, kind="ExternalInput")
    d_b = nc.dram_tensor("bias", [64, 6], F32, kind="ExternalInput")  # b0,b0n,b1,b1n,b2,b2n
    d_blp = nc.dram_tensor("blp", [128, 3], F32, kind="ExternalInput")  # 288 perm bias, col-chunks
    d_eye = nc.dram_tensor("eye", [128, 32], BF16, kind="ExternalInput")
    d_out = nc.dram_tensor("out", [32, H, H], F32, kind="ExternalOutput")
    if debug:
        d_q1 = nc.dram_tensor("dbg_q1", [128, NCOL], F32, kind="ExternalOutput")
        d_q2 = nc.dram_tensor("dbg_q2", [128, NCOL], F32, kind="ExternalOutput")
        d_q3 = nc.dram_tensor("dbg_q3", [128, NCOL], F32, kind="ExternalOutput")

    strips = _strips()

    with tile.TileContext(nc) as tc, ExitStack() as ctx:
        wp = ctx.enter_context(tc.tile_pool(name="wp", bufs=1))
        big = ctx.enter_context(tc.tile_pool(name="big", bufs=1))
        xs = ctx.enter_context(tc.tile_pool(name="xs", bufs=2))
        sm = ctx.enter_context(tc.tile_pool(name="sm", bufs=3))
        fu = ctx.enter_context(tc.tile_pool(name="fu", bufs=2))
        pA = ctx.enter_context(tc.tile_pool(name="pA", bufs=2, space="PSUM"))
        pB = ctx.enter_context(tc.tile_pool(name="pB", bufs=2, space="PSUM"))
        pC = ctx.enter_context(tc.tile_pool(name="pC", bufs=2, space="PSUM"))
        pD = ctx.enter_context(tc.tile_pool(name="pD", bufs=2, space="PSUM"))

        # ---- weights / constants to SBUF ----
        w0c1 = wp.tile([128, 9 * 64], BF16)
        w0c2 = wp.tile([96, 3 * 64], BF16)
        w1P = wp.tile([128, 3 * 64], BF16)
        w1S = wp.tile([64, 3 * 64], BF16)
        w2P = wp.tile([128, 3 * 64], BF16)
        w2S = wp.tile([64, 3 * 64], BF16)
        wlP = wp.tile([128, 3 * 288], BF16)
        wlS = wp.tile([64, 3 * 288], BF16)
        bias = wp.tile([64, 6], F32)
        blp = wp.tile([128, 3], F32)
        eye = wp.tile([128, 32], BF16)
        for dst, src in ((w0c1, d_w0c1), (w0c2, d_w0c2), (w1P, d_w1P),
                         (w1S, d_w1S), (w2P, d_w2P), (w2S, d_w2S),
                         (wlP, d_wlP), (wlS, d_wlS), (eye, d_eye)):
            nc.sync.dma_start(out=dst[:], in_=src.ap())
        for dst, src in ((bias, d_b), (blp, d_blp)):
            nc.sync.dma_start(out=dst[:], in_=src.ap())

        def wslice(wt, i, m0, mw, step=64):
            # [K, mw] slice for matmul lhsT: tap/dc index i, out-ch offset m0
            return wt[:, i * step + m0: i * step + m0 + mw]

        def r_(t):
            return t

        # ---- big feature tiles (two slots: A holds q1 then q3, B holds q2) ----
        def new_q(tag):
            q = big.tile([128, NCOL], BF16, tag=tag)
            # zero the pad structure (lower half: head, inter-row cells, tail;
            # upper half: head cell + tail region never covered by upcopies)
            nc.gpsimd.memset(q[0:64, 0:130], 0.0)
            inter = q[0:64, 258:258 + 127 * ST].rearrange(
                "p (m s) -> p m s", s=ST)[:, :, 0:1]
            nc.gpsimd.memset(inter, 0.0)
            nc.gpsimd.memset(q[0:64, ST * 129:NCOL], 0.0)
            nc.gpsimd.memset(q[64:128, 0:1], 0.0)
            last_up = _j0(strips[-1][0]) - ST + strips[-1][1] * ST
            nc.gpsimd.memset(q[64:128, last_up:NCOL], 0.0)
            return q

        def evac_conv(ps, q, j0, nr, n, bcol):
            # leaky-relu from psum into q's valid cells + shifted upper copy
            rn = sm.tile([64, 3 * ST], F32, tag="rn")
            nc.scalar.activation(rn[:, 0:n], ps[:, 0:n], ACTF.Relu,
                                 bias=bias[:, bcol + 1:bcol + 2], scale=-0.99)
            src = ps[:, 0:n].rearrange("p (r c) -> p r c", c=ST)[:, :, 0:128]
            rnv = rn[:, 0:n].rearrange("p (r c) -> p r c", c=ST)[:, :, 0:128]
            dst = q[0:64, j0:j0 + n].rearrange("p (r c) -> p r c", c=ST)[:, :, 0:128]
            nc.vector.scalar_tensor_tensor(dst, src,
                                           bias[:, bcol:bcol + 1], rnv,
                                           op0=ALU.add, op1=ALU.add)
            # upper K-stack copy: up[j] = q[j+129] over this strip's window
            nc.sync.dma_start(out=q[64:128, j0 - ST:j0 - ST + n],
                              in_=q[0:64, j0:j0 + n])

        # ================= conv0 (streamed input strips) =================
        q1 = new_q("A")
        gi = 0
        while gi < len(strips):
            grp = strips[gi:gi + X_GROUP_STRIPS]
            r0g = grp[0][0]
            nrg = sum(nr for _, nr in grp)
            jg = _j0(r0g)
            win = ST * nrg + 260
            x1 = xs.tile([128, ST * 9 + 260], BF16, tag="x1")
            x2 = xs.tile([96, ST * 9 + 260], BF16, tag="x2")
            nc.sync.dma_start(out=x1[:, 0:win], in_=d_xpad.ap()[:, jg - 130:jg - 130 + win])
            for k in range(3):
                nc.sync.dma_start(
                    out=x2[32 * k:32 * k + 32, 0:win],
                    in_=d_mupad.ap()[:, jg - 130 + ST * k:jg - 130 + ST * k + win])
            for (r0, nr) in grp:
                j0 = _j0(r0)
                n = ST * nr
                loc = j0 - jg + 130
                ps = pA.tile([64, 3 * ST], F32, tag="pA")
                first = True
                for t, (dr, dc) in enumerate([(a, b) for a in (-1, 0, 1) for b in (-1, 0, 1)]):
                    o = loc + ST * dr + dc
                    nc.tensor.matmul(ps[:, 0:n], r_(wslice(w0c1, t, 0, 64)),
                                     r_(x1[:, o:o + n]), start=first, stop=False)
                    first = False
                for i, dc in enumerate((-1, 0, 1)):
                    o = loc - ST + dc
                    nc.tensor.matmul(ps[:, 0:n], r_(wslice(w0c2, i, 0, 64)),
                                     r_(x2[:, o:o + n]), start=False, stop=(i == 2))
                evac_conv(ps, q1, j0, nr, n, 0)
            gi += X_GROUP_STRIPS
        if debug:
            nc.sync.dma_start(out=d_q1.ap(), in_=q1[:])

        # ================= conv1 / conv2 =================
        def mid_conv(qin, qout, wP, wS, bcol):
            for (r0, nr) in strips:
                j0 = _j0(r0)
                n = ST * nr
                ps = pA.tile([64, 3 * ST], F32, tag="pA")
                for i, dc in enumerate((-1, 0, 1)):
                    o = j0 - ST + dc
                    nc.tensor.matmul(ps[:, 0:n], r_(wslice(wP, i, 0, 64)),
                                     r_(qin[0:128, o:o + n]), start=(i == 0), stop=False)
                for i, dc in enumerate((-1, 0, 1)):
                    o = j0 + ST + dc
                    nc.tensor.matmul(ps[:, 0:n], r_(wslice(wS, i, 0, 64)),
                                     r_(qin[0:64, o:o + n]), start=False, stop=(i == 2))
                evac_conv(ps, qout, j0, nr, n, bcol)

        q2 = new_q("B")
        mid_conv(q1, q2, w1P, w1S, 2)
        if debug:
            nc.sync.dma_start(out=d_q2.ap(), in_=q2[:])

        q3 = new_q("A")
        mid_conv(q2, q3, w2P, w2S, 4)
        if debug:
            nc.sync.dma_start(out=d_q3.ap(), in_=q3[:])

        # ================= conv_last + softmax + fusion =================
        for (r0, nr) in strips:
            j0 = _j0(r0)
            n = ST * nr
            ph = [pA.tile([128, 3 * ST], F32, tag="pA", name="ph0"),
                  pB.tile([128, 3 * ST], F32, tag="pB", name="ph1"),
                  pC.tile([32, 3 * ST], F32, tag="pC", name="ph2")]
            for ci, (m0, mw) in enumerate(((0, 128), (128, 128), (256, 32))):
                ps = ph[ci]
                for i, dc in enumerate((-1, 0, 1)):
                    o = j0 - ST + dc
                    nc.tensor.matmul(ps[:, 0:n], r_(wslice(wlP, i, m0, mw, 288)),
                                     r_(q3[0:128, o:o + n]), start=(i == 0), stop=False)
                for i, dc in enumerate((-1, 0, 1)):
                    o = j0 + ST + dc
                    nc.tensor.matmul(ps[:, 0:n], r_(wslice(wlS, i, m0, mw, 288)),
                                     r_(q3[0:64, o:o + n]), start=False, stop=(i == 2))
            # exp(head + b_last)
            ea = fu.tile([128, 3 * ST], BF16, tag="ea")
            eb = fu.tile([128, 3 * ST], BF16, tag="eb")
            ec = fu.tile([32, 3 * ST], BF16, tag="ec")
            nc.scalar.activation(ea[:, 0:n], ph[0][:, 0:n], ACTF.Exp, bias=blp[:, 0:1])
            nc.scalar.activation(eb[:, 0:n], ph[1][:, 0:n], ACTF.Exp, bias=blp[:, 1:2])
            nc.scalar.activation(ec[:, 0:n], ph[2][:, 0:n], ACTF.Exp, bias=blp[0:32, 2:3])
            # patch strips of mem_stab (xpad rows 96:128), z strip (rows 64:96)
            msa = fu.tile([128, 3 * ST], BF16, tag="msa")
            msb = fu.tile([128, 3 * ST], BF16, tag="msb")
            msc = fu.tile([32, 3 * ST], BF16, tag="msc")
            for g in range(4):
                nc.sync.dma_start(out=msa[32 * g:32 * g + 32, 0:n],
                                  in_=d_xpad.ap()[96:128, j0 + P_TAPS[g]:j0 + P_TAPS[g] + n])
                nc.sync.dma_start(out=msb[32 * g:32 * g + 32, 0:n],
                                  in_=d_xpad.ap()[96:128, j0 + P_TAPS[4 + g]:j0 + P_TAPS[4 + g] + n])
            nc.sync.dma_start(out=msc[:, 0:n],
                              in_=d_xpad.ap()[96:128, j0 + P_TAPS[8]:j0 + P_TAPS[8] + n])
            rhs3 = fu.tile([64, 3 * ST], BF16, tag="rhs3")
            nc.sync.dma_start(out=rhs3[32:64, 0:n], in_=d_xpad.ap()[64:96, j0:j0 + n])
            ta = fu.tile([128, 3 * ST], BF16, tag="ta")
            tb = fu.tile([128, 3 * ST], BF16, tag="tb")
            nc.vector.tensor_mul(ta[:, 0:n], ea[:, 0:n], msa[:, 0:n])
            nc.vector.tensor_mul(tb[:, 0:n], eb[:, 0:n], msb[:, 0:n])
            nc.vector.tensor_mul(rhs3[0:32, 0:n], ec[:, 0:n], msc[:, 0:n])
            # numerator (psum 0:32) and denominator (psum 32:64)
            nd = pD.tile([64, 3 * ST], F32, tag="pD")
            nc.tensor.matmul(nd[0:32, 0:n], r_(eye[:]), r_(ta[:, 0:n]), start=True, stop=False)
            nc.tensor.matmul(nd[0:32, 0:n], r_(eye[:]), r_(tb[:, 0:n]), start=False, stop=False)
            nc.tensor.matmul(nd[0:32, 0:n], r_(eye[0:64, :]), r_(rhs3[:, 0:n]), start=False, stop=True)
            nc.tensor.matmul(nd[32:64, 0:n], r_(eye[:]), r_(ea[:, 0:n]), start=True, stop=False)
            nc.tensor.matmul(nd[32:64, 0:n], r_(eye[:]), r_(eb[:, 0:n]), start=False, stop=False)
            nc.tensor.matmul(nd[32:64, 0:n], r_(eye[0:32, :]), r_(ec[:, 0:n]), start=False, stop=True)
            den = fu.tile([32, 3 * ST], F32, tag="den")
            rde = fu.tile([32, 3 * ST], F32, tag="rde")
            ost = fu.tile([32, 3 * ST], F32, tag="ost")
            nc.vector.tensor_scalar_add(den[:, 0:n], nd[32:64, 0:n], 1.0)
            nc.vector.reciprocal_approx_fast(rde[:, 0:n], den[:, 0:n])
            nc.vector.tensor_mul(ost[:, 0:n], nd[0:32, 0:n], rde[:, 0:n])
            src = ost[:, 0:n].rearrange("p (r c) -> p r c", c=ST)[:, :, 0:128]
            nc.sync.dma_start(out=d_out.ap()[:, r0:r0 + nr, :], in_=src)

    nc.compile()
    return nc


BF16_NP = mybir.dt.np(mybir.dt.bfloat16)


def _pad_rows(x, cols):
    # x: [C, 128, 128] -> zero-padded flat rows [C, cols], bf16
    c = x.shape[0]
    buf = np.zeros((c, cols), dtype=BF16_NP)
    buf[:, 130:130 + ST * 128].reshape(c, 128, ST)[:, :, 0:128] = x.astype(BF16_NP)
    return buf


def _prep_shared(w0, b0, w1, b1, w2, b2, w_last, b_last):
    f = np.float32
    w0t = np.transpose(np.asarray(w0, f), (1, 2, 3, 0))      # [160,3,3,64]
    w0c1 = np.ascontiguousarray(w0t[0:128].reshape(128, 9 * 64))
    w0c2 = np.ascontiguousarray(
        np.transpose(w0t[128:160], (1, 0, 2, 3)).reshape(96, 3 * 64))
    def mid(w):
        wt = np.transpose(np.asarray(w, f), (1, 2, 3, 0))    # [64,3,3,64]
        wP = np.ascontiguousarray(
            np.concatenate([wt[:, 0], wt[:, 1]], 0).reshape(128, 3 * 64))
        wS = np.ascontiguousarray(wt[:, 2].reshape(64, 3 * 64))
        return wP, wS
    w1P, w1S = mid(w1)
    w2P, w2S = mid(w2)
    perm = np.array([(pp % 32) * 9 + pp // 32 for pp in range(288)])
    wl2 = np.asarray(w_last, f)[perm]                        # [288,64,3,3] p-major
    wlt = np.transpose(wl2, (1, 2, 3, 0))                    # [64,3,3,288]
    wlP = np.ascontiguousarray(
        np.concatenate([wlt[:, 0], wlt[:, 1]], 0).reshape(128, 3 * 288))
    wlS = np.ascontiguousarray(wlt[:, 2].reshape(64, 3 * 288))
    bias = np.stack([np.asarray(b0, f), -0.99 * np.asarray(b0, f),
                     np.asarray(b1, f), -0.99 * np.asarray(b1, f),
                     np.asarray(b2, f), -0.99 * np.asarray(b2, f)], axis=1)
    blp_flat = np.asarray(b_last, f)[perm]
    blp = np.zeros((128, 3), f)
    blp[:, 0] = blp_flat[0:128]
    blp[:, 1] = blp_flat[128:256]
    blp[0:32, 2] = blp_flat[256:288]
    eye = np.tile(np.eye(32, dtype=f), (4, 1))
    out = dict(w0c1=w0c1, w0c2=w0c2, w1P=w1P, w1S=w1S, w2P=w2P, w2S=w2S,
               wlP=wlP, wlS=wlS, eye=eye)
    out = {k: v.astype(BF16_NP) for k, v in out.items()}
    out["bias"] = np.ascontiguousarray(bias)
    out["blp"] = blp
    return out


_NC_CACHE = {}


def _get_nc(debug=False):
    if debug not in _NC_CACHE:
        _NC_CACHE[debug] = _build_program(debug)
    return _NC_CACHE[debug]


def make_in_maps(z, backbone, mem_stab, mem_unstab, shared):
    f = np.float32
    z = np.asarray(z, f); backbone = np.asarray(backbone, f)
    ms = np.asarray(mem_stab, f); mu = np.asarray(mem_unstab, f)
    maps = []
    for b in range(z.shape[0]):
        x160 = np.concatenate([backbone[b], z[b], ms[b]], axis=0)  # [128,...]
        maps.append(dict(xpad=_pad_rows(x160, XCOL),
                         mupad=_pad_rows(mu[b], MUCOL), **shared))
    return maps


def kernel(z, backbone, mem_stab, mem_unstab, w0, b0, w1, b1, w2, b2,
           w_last, b_last, fusion_kernel_size):
    assert int(fusion_kernel_size) == 3
    shared = _prep_shared(w0, b0, w1, b1, w2, b2, w_last, b_last)
    in_maps = make_in_maps(z, backbone, mem_stab, mem_unstab, shared)
    nc = _get_nc()
    res = run_bass_kernel_spmd(nc, in_maps, core_ids=list(range(len(in_maps))))
    out = np.stack([r["out"] for r in res.results], axis=0)
    return out.astype(np.float32)



# revision 12
# speedup vs baseline: 1.4904x; 1.4904x over previous
"""Trainium2 Bass kernel for nn_ControlledConvEMAStabilizer.

Pipeline (per batch image, one NeuronCore each, batch-parallel over 8 cores):
  q = cat(backbone, z, mem_stab, mem_unstab)          # 160ch
  q = lrelu(conv3x3(q, w0) + b0)                      # -> 64ch
  q = lrelu(conv3x3(q, w1) + b1)                      # -> 64ch
  q = lrelu(conv3x3(q, w2) + b2)                      # -> 64ch
  head = conv3x3(q, w_last) + b_last                  # -> 288ch = 9 taps x 32ch
  eta  = softmax([head; 0]) over the 9+1 slots
  out  = sum_p unfold(mem_stab)[p] * eta[p] + eta[9] * z

Implementation notes:
  - Feature maps live in SBUF as zero-padded flat rows: image pixel (r,c) at
    column 129*(r+1)+1+c (row stride 129, shared single pad column between
    rows, one pad row top/bottom).  Every 3x3 tap is a pure column offset,
    so convs are PSUM-accumulated matmuls over shifted views.
  - K-stacking: intermediates stored twice in one [128, NCOL] tile:
    partitions 0:64 = q, partitions 64:128 = q shifted +129 (one image row).
    A K=128 matmul applies two vertical taps at once.
  - PE sub-array packing via tile_position: strips processed in PAIRS.
    M=64 convs (conv0/1/2) run both strips' matmuls concurrently in the two
    column halves of the PE array (out partitions 0:64 / 64:128).  conv_last
    K=64 tap matmuls row-pair across strips (rows 0:64 / 64:128); the M=32
    head chunk and the softmax-reduction matmuls pack 2- and 4-wide into
    32-column groups.  Measured ~1.8-4x PE throughput vs serial.
  - LeakyReLU evac: single ScalarE activation (Lrelu, alpha=0.01, bias) from
    PSUM into q's strided pixel cells; K-stack upcopy via gpsimd-issued DMA.
  - Fusion tail: exp on ACT, eta*patch products on DVE against host-prepared
    pre-shifted mem_stab tap stacks resident in SBUF (no per-strip DMA),
    partition-group sums via 4-wide packed identity matmuls, recip+mul DVE.
"""

import numpy as np
from contextlib import ExitStack

import concourse.bacc as bacc
import concourse.tile as tile
from concourse import mybir
from concourse.bass_utils import run_bass_kernel_spmd

F32 = mybir.dt.float32
BF16 = mybir.dt.bfloat16
ALU = mybir.AluOpType
ACTF = mybir.ActivationFunctionType

H = 128
ST = 129                      # padded row stride
NCOL = ST * 130 + 2           # 16772 sbuf cols
MUCOL = NCOL + 2 * ST + 2
RPS = 3                       # rows per strip

# taps in fusion/unfold order p = 3*kh + kw -> offset 129*(kh-1) + (kw-1)
P_TAPS = [ST * (kh - 1) + (kw - 1) for kh in range(3) for kw in range(3)]


def _j0(r0):
    return ST * (r0 + 1) + 1


def _strips():
    out, r0 = [], 0
    while r0 < H:
        nr = min(RPS, H - r0)
        out.append((r0, nr))
        r0 += nr
    return out


def _pairs():
    s = _strips()
    out, i = [], 0
    while i < len(s):
        if i + 1 < len(s) and s[i + 1][1] == RPS:
            out.append((s[i], s[i + 1]))
            i += 2
        else:
            out.append((s[i],))
            i += 1
    return out


def _build_program(debug=False):
    nc = bacc.Bacc("TRN2", target_bir_lowering=False, debug=False)

    d_xpad = nc.dram_tensor("xpad", [128, NCOL], BF16, kind="ExternalInput")
    d_mu3 = nc.dram_tensor("mu3", [96, MUCOL], BF16, kind="ExternalInput")
    d_msa = nc.dram_tensor("msa", [128, NCOL], BF16, kind="ExternalInput")
    d_msb = nc.dram_tensor("msb", [128, NCOL], BF16, kind="ExternalInput")
    d_ms8 = nc.dram_tensor("ms8", [32, NCOL], BF16, kind="ExternalInput")
    d_w0c1 = nc.dram_tensor("w0c1", [128, 9 * 64], BF16, kind="ExternalInput")
    d_w0c2 = nc.dram_tensor("w0c2", [96, 3 * 64], BF16, kind="ExternalInput")
    d_w1P = nc.dram_tensor("w1P", [128, 3 * 64], BF16, kind="ExternalInput")
    d_w1S = nc.dram_tensor("w1S", [64, 3 * 64], BF16, kind="ExternalInput")
    d_w2P = nc.dram_tensor("w2P", [128, 3 * 64], BF16, kind="ExternalInput")
    d_w2S = nc.dram_tensor("w2S", [64, 3 * 64], BF16, kind="ExternalInput")
    d_wlP = nc.dram_tensor("wlP", [128, 3 * 288], BF16, kind="ExternalInput")
    d_wlS2 = nc.dram_tensor("wlS2", [128, 3 * 288], BF16, kind="ExternalInput")
    d_b = nc.dram_tensor("bias", [128, 3], F32, kind="ExternalInput")
    d_blp = nc.dram_tensor("blp", [128, 4], F32, kind="ExternalInput")
    d_eye = nc.dram_tensor("eye", [128, 32], BF16, kind="ExternalInput")
    d_out = nc.dram_tensor("out", [32, H, H], F32, kind="ExternalOutput")
    if debug:
        d_q1 = nc.dram_tensor("dbg_q1", [128, NCOL], F32, kind="ExternalOutput")
        d_q2 = nc.dram_tensor("dbg_q2", [128, NCOL], F32, kind="ExternalOutput")
        d_q3 = nc.dram_tensor("dbg_q3", [128, NCOL], F32, kind="ExternalOutput")

    pairs = _pairs()
    strips = _strips()

    with tile.TileContext(nc) as tc, ExitStack() as ctx:
        wp = ctx.enter_context(tc.tile_pool(name="wp", bufs=1))
        big = ctx.enter_context(tc.tile_pool(name="big", bufs=1))
        xs = ctx.enter_context(tc.tile_pool(name="xs", bufs=2))
        fu = ctx.enter_context(tc.tile_pool(name="fu", bufs=2))
        f1 = ctx.enter_context(tc.tile_pool(name="f1", bufs=1))
        pm = ctx.enter_context(tc.tile_pool(name="pm", bufs=2, space="PSUM"))
        pA = ctx.enter_context(tc.tile_pool(name="pA", bufs=2, space="PSUM"))
        pB = ctx.enter_context(tc.tile_pool(name="pB", bufs=2, space="PSUM"))
        pC = ctx.enter_context(tc.tile_pool(name="pC", bufs=1, space="PSUM"))
        pD = ctx.enter_context(tc.tile_pool(name="pD", bufs=1, space="PSUM"))

        # ---- weights / constants / resident stacks to SBUF ----
        w0c1 = wp.tile([128, 9 * 64], BF16)
        w0c2 = wp.tile([96, 3 * 64], BF16)
        w1P = wp.tile([128, 3 * 64], BF16)
        w1S = wp.tile([64, 3 * 64], BF16)
        w2P = wp.tile([128, 3 * 64], BF16)
        w2S = wp.tile([64, 3 * 64], BF16)
        wlP = wp.tile([128, 3 * 288], BF16)
        wlS2 = wp.tile([128, 3 * 288], BF16)
        bias = wp.tile([128, 3], F32)
        blp = wp.tile([128, 4], F32)
        eye = wp.tile([128, 32], BF16)
        msa = wp.tile([128, NCOL], BF16)
        msb = wp.tile([128, NCOL], BF16)
        ms8 = wp.tile([32, NCOL], BF16)
        for dst, src in ((w0c1, d_w0c1), (w0c2, d_w0c2), (w1P, d_w1P),
                         (w1S, d_w1S), (w2P, d_w2P), (w2S, d_w2S),
                         (wlP, d_wlP), (wlS2, d_wlS2), (eye, d_eye),
                         (bias, d_b), (blp, d_blp)):
            nc.sync.dma_start(out=dst[:], in_=src.ap())
        for dst, src in ((msa, d_msa), (msb, d_msb), (ms8, d_ms8)):
            nc.scalar.dma_start(out=dst[:], in_=src.ap())

        def wsl(wt, i, m0, mw, step=64):
            return wt[:, i * step + m0: i * step + m0 + mw]

        def new_q(tag):
            q = big.tile([128, NCOL], BF16, tag=tag)
            # zero the pad structure (lower half: head, inter-row cells, tail;
            # upper half: head cell + tail region never covered by upcopies)
            nc.gpsimd.memset(q[0:64, 0:130], 0.0)
            inter = q[0:64, 258:258 + 127 * ST].rearrange(
                "p (m s) -> p m s", s=ST)[:, :, 0:1]
            nc.gpsimd.memset(inter, 0.0)
            nc.gpsimd.memset(q[0:64, ST * 129:NCOL], 0.0)
            last_j0, last_n = _j0(strips[-1][0]), strips[-1][1] * ST
            nc.gpsimd.memset(q[64:128, 0:1], 0.0)
            nc.gpsimd.memset(q[64:128, last_j0 - ST + last_n:NCOL], 0.0)
            return q

        def evac(ps, q, pr, bcol):
            # ps[64*i : 64*i+64] holds strip i's 64ch; lrelu + pad-keep write
            for i, (r0, nr) in enumerate(pr):
                j0, n = _j0(r0), ST * nr
                src = ps[64 * i:64 * i + 64, 0:n].rearrange(
                    "p (r c) -> p r c", c=ST)[:, :, 0:128]
                dst = q[0:64, j0:j0 + n].rearrange(
                    "p (r c) -> p r c", c=ST)[:, :, 0:128]
                nc.scalar.activation(dst, src, ACTF.Lrelu,
                                     bias=bias[64 * i:64 * i + 64, bcol:bcol + 1],
                                     alpha=0.01)
                nc.gpsimd.dma_start(out=q[64:128, j0 - ST:j0 - ST + n],
                                    in_=q[0:64, j0:j0 + n])

        # ================= conv0 (streamed input, strip-pair groups) ======
        q1 = new_q("A")
        for pr in pairs:
            r0g = pr[0][0]
            nrg = sum(nr for _, nr in pr)
            jg = _j0(r0g)
            win = ST * nrg + 260
            x1 = xs.tile([128, ST * 6 + 260], BF16, tag="x1")
            x2 = xs.tile([96, ST * 6 + 260], BF16, tag="x2")
            nc.sync.dma_start(out=x1[:, 0:win],
                              in_=d_xpad.ap()[:, jg - 130:jg - 130 + win])
            nc.sync.dma_start(out=x2[:, 0:win],
                              in_=d_mu3.ap()[:, jg - 130:jg - 130 + win])
            ps = pm.tile([128, 3 * ST], F32, tag="pm")
            off = [(a, b) for a in (-1, 0, 1) for b in (-1, 0, 1)]
            for t, (dr, dc) in enumerate(off):
                for i, (r0, nr) in enumerate(pr):
                    loc = _j0(r0) - jg + 130
                    o = loc + ST * dr + dc
                    nc.tensor.matmul(ps[64 * i:64 * i + 64, 0:ST * nr],
                                     wsl(w0c1, t, 0, 64), x1[:, o:o + ST * nr],
                                     start=(t == 0), stop=False)
            for t, dc in enumerate((-1, 0, 1)):
                for i, (r0, nr) in enumerate(pr):
                    loc = _j0(r0) - jg + 130
                    o = loc - ST + dc
                    nc.tensor.matmul(ps[64 * i:64 * i + 64, 0:ST * nr],
                                     wsl(w0c2, t, 0, 64), x2[:, o:o + ST * nr],
                                     start=False, stop=(t == 2))
            evac(ps, q1, pr, 0)
        if debug:
            nc.sync.dma_start(out=d_q1.ap(), in_=q1[:])

        # ================= conv1 / conv2 =================
        def mid_conv(qin, qout, wP, wS, bcol):
            for pr in pairs:
                ps = pm.tile([128, 3 * ST], F32, tag="pm")
                for t, dc in enumerate((-1, 0, 1)):
                    for i, (r0, nr) in enumerate(pr):
                        o = _j0(r0) - ST + dc
                        nc.tensor.matmul(ps[64 * i:64 * i + 64, 0:ST * nr],
                                         wsl(wP, t, 0, 64), qin[0:128, o:o + ST * nr],
                                         start=(t == 0), stop=False)
                for t, dc in enumerate((-1, 0, 1)):
                    for i, (r0, nr) in enumerate(pr):
                        o = _j0(r0) + ST + dc
                        nc.tensor.matmul(ps[64 * i:64 * i + 64, 0:ST * nr],
                                         wsl(wS, t, 0, 64), qin[0:64, o:o + ST * nr],
                                         start=False, stop=(t == 2))
                evac(ps, qout, pr, bcol)

        q2 = new_q("B")
        mid_conv(q1, q2, w1P, w1S, 1)
        if debug:
            nc.sync.dma_start(out=d_q2.ap(), in_=q2[:])
        q3 = new_q("A")
        mid_conv(q2, q3, w2P, w2S, 2)
        if debug:
            nc.sync.dma_start(out=d_q3.ap(), in_=q3[:])

        # ================= conv_last + softmax + fusion =================
        for pr in pairs:
            np_ = len(pr)
            j0s = [_j0(r0) for r0, _ in pr]
            ns = [ST * nr for _, nr in pr]
            # z loads (no deps -> issue early)
            rz = [fu.tile([64, 3 * ST], BF16, tag=f"rz{i}", name=f"rz{i}")
                  for i in range(np_)]
            for i in range(np_):
                nc.sync.dma_start(out=rz[i][32:64, 0:ns[i]],
                                  in_=d_xpad.ap()[64:96, j0s[i]:j0s[i] + ns[i]])
            ppool = (pA, pB)
            ea, eb, ec = [], [], []
            # chunk 0 (head channels 0:128) then chunk 1 (128:256)
            for ck, (m0, edst) in enumerate(((0, ea), (128, eb))):
                ph = [ppool[i].tile([128, 3 * ST], F32, tag=f"p{'AB'[i]}",
                                    name=f"ph{i}")
                      for i in range(np_)]
                for t, dc in enumerate((-1, 0, 1)):
                    for i in range(np_):
                        o = j0s[i] - ST + dc
                        nc.tensor.matmul(ph[i][:, 0:ns[i]],
                                         wsl(wlP, t, m0, 128, 288),
                                         q3[0:128, o:o + ns[i]],
                                         start=(t == 0), stop=False)
                for t, dc in enumerate((-1, 0, 1)):
                    # row-paired K=64 taps: strip0 rows 0:64, strip1 rows 64:128
                    for i in range(np_):
                        if i == 0:
                            lhs = wsl(wlS2, t, m0, 128, 288)[0:64]
                            rhs = q3[0:64, j0s[0] + ST + dc:j0s[0] + ST + dc + ns[0]]
                        else:
                            lhs = wsl(wlS2, t, m0, 128, 288)[64:128]
                            rhs = q3[64:128, j0s[1] + dc:j0s[1] + dc + ns[1]]
                        nc.tensor.matmul(ph[i][:, 0:ns[i]], lhs, rhs,
                                         start=False, stop=(t == 2))
                for i in range(np_):
                    e = fu.tile([128, 3 * ST], BF16, tag=f"e{ck}{i}")
                    nc.scalar.activation(e[:, 0:ns[i]], ph[i][:, 0:ns[i]],
                                         ACTF.Exp, bias=blp[:, ck:ck + 1])
                    edst.append(e)
            # chunk 2 (M=32, both strips col-packed into one [64,.] psum)
            phc = pC.tile([64, 3 * ST], F32, tag="pC")
            for t, dc in enumerate((-1, 0, 1)):
                for i in range(np_):
                    o = j0s[i] - ST + dc
                    nc.tensor.matmul(phc[32 * i:32 * i + 32, 0:ns[i]],
                                     wsl(wlP, t, 256, 32, 288),
                                     q3[0:128, o:o + ns[i]],
                                     start=(t == 0), stop=False)
            for t, dc in enumerate((-1, 0, 1)):
                for i in range(np_):
                    o = j0s[i] + ST + dc
                    nc.tensor.matmul(phc[32 * i:32 * i + 32, 0:ns[i]],
                                     wsl(wlS2, t, 256, 32, 288)[0:64],
                                     q3[0:64, o:o + ns[i]],
                                     start=False, stop=(t == 2))
            for i in range(np_):
                e = fu.tile([32, 3 * ST], BF16, tag=f"ec{i}")
                nc.scalar.activation(e[:, 0:ns[i]], phc[32 * i:32 * i + 32, 0:ns[i]],
                                     ACTF.Exp, bias=blp[32 * i:32 * i + 32, 3:4])
                ec.append(e)
            # eta * patch products (DVE, same-base operands)
            ta, tb = [], []
            for i in range(np_):
                t1 = fu.tile([128, 3 * ST], BF16, tag=f"ta{i}")
                t2 = fu.tile([128, 3 * ST], BF16, tag=f"tb{i}")
                nc.vector.tensor_mul(t1[:, 0:ns[i]], ea[i][:, 0:ns[i]],
                                     msa[:, j0s[i]:j0s[i] + ns[i]])
                nc.vector.tensor_mul(t2[:, 0:ns[i]], eb[i][:, 0:ns[i]],
                                     msb[:, j0s[i]:j0s[i] + ns[i]])
                nc.vector.tensor_mul(rz[i][0:32, 0:ns[i]], ec[i][:, 0:ns[i]],
                                     ms8[:, j0s[i]:j0s[i] + ns[i]])
                ta.append(t1)
                tb.append(t2)
            # packed reduction matmuls: num strip i -> nd[32i:32i+32],
            # den strip i -> nd[64+32i : 96+32i]
            nd = pD.tile([128, 3 * ST], F32, tag="pD")
            for t in range(3):
                for i in range(np_):
                    npos = 32 * i
                    dpos = 64 + 32 * i
                    nl, nr_ = ((eye[:], ta[i]), (eye[:], tb[i]),
                               (eye[0:64], rz[i]))[t]
                    dl, dr = ((eye[:], ea[i]), (eye[:], eb[i]),
                              (eye[0:32], ec[i]))[t]
                    nc.tensor.matmul(nd[npos:npos + 32, 0:ns[i]], nl,
                                     nr_[:, 0:ns[i]], start=(t == 0),
                                     stop=(t == 2), tile_position=(0, npos))
                    nc.tensor.matmul(nd[dpos:dpos + 32, 0:ns[i]], dl,
                                     dr[:, 0:ns[i]], start=(t == 0),
                                     stop=(t == 2), tile_position=(0, dpos))
            # rde = 1/(den+1); ost = num * rde  (both strips at once)
            w = 32 * np_
            den = f1.tile([64, 3 * ST], F32, tag="den")
            rde = f1.tile([64, 3 * ST], F32, tag="rde")
            ost = f1.tile([64, 3 * ST], F32, tag="ost")
            nmax = max(ns)
            nc.vector.tensor_scalar_add(den[0:w, 0:nmax], nd[64:64 + w, 0:nmax], 1.0)
            nc.vector.reciprocal_approx_fast(rde[0:w, 0:nmax], den[0:w, 0:nmax])
            nc.vector.tensor_mul(ost[0:w, 0:nmax], nd[0:w, 0:nmax], rde[0:w, 0:nmax])
            for i, (r0, nr) in enumerate(pr):
                src = ost[32 * i:32 * i + 32, 0:ns[i]].rearrange(
                    "p (r c) -> p r c", c=ST)[:, :, 0:128]
                nc.sync.dma_start(out=d_out.ap()[:, r0:r0 + nr, :], in_=src)

    nc.compile()
    return nc


BF16_NP = mybir.dt.np(mybir.dt.bfloat16)


def _pad_rows(x, cols):
    # x: [C, 128, 128] -> zero-padded flat rows [C, cols], bf16
    c = x.shape[0]
    buf = np.zeros((c, cols), dtype=BF16_NP)
    buf[:, 130:130 + ST * 128].reshape(c, 128, ST)[:, :, 0:128] = x.astype(BF16_NP)
    return buf


def _shift_stack(flat, offs):
    # flat: [32, NCOL]; returns [32*len(offs), NCOL] rows shifted by offs
    ext = np.zeros((flat.shape[0], NCOL + 264), dtype=flat.dtype)
    ext[:, 132:132 + NCOL] = flat
    return np.concatenate([ext[:, 132 + o:132 + o + NCOL] for o in offs], axis=0)


def _prep_shared(w0, b0, w1, b1, w2, b2, w_last, b_last):
    f = np.float32
    w0t = np.transpose(np.asarray(w0, f), (1, 2, 3, 0))      # [160,3,3,64]
    w0c1 = np.ascontiguousarray(w0t[0:128].reshape(128, 9 * 64))
    w0c2 = np.ascontiguousarray(
        np.transpose(w0t[128:160], (1, 0, 2, 3)).reshape(96, 3 * 64))

    def mid(w):
        wt = np.transpose(np.asarray(w, f), (1, 2, 3, 0))    # [64,3,3,64]
        wP = np.ascontiguousarray(
            np.concatenate([wt[:, 0], wt[:, 1]], 0).reshape(128, 3 * 64))
        wS = np.ascontiguousarray(wt[:, 2].reshape(64, 3 * 64))
        return wP, wS

    w1P, w1S = mid(w1)
    w2P, w2S = mid(w2)
    perm = np.array([(pp % 32) * 9 + pp // 32 for pp in range(288)])
    wl2 = np.asarray(w_last, f)[perm]                        # [288,64,3,3]
    wlt = np.transpose(wl2, (1, 2, 3, 0))                    # [64,3,3,288]
    wlP = np.ascontiguousarray(
        np.concatenate([wlt[:, 0], wlt[:, 1]], 0).reshape(128, 3 * 288))
    wlS = np.ascontiguousarray(wlt[:, 2].reshape(64, 3 * 288))
    wlS2 = np.concatenate([wlS, wlS], axis=0)                # [128, 864]
    bias = np.stack([np.asarray(b0, f), np.asarray(b1, f),
                     np.asarray(b2, f)], axis=1)             # [64, 3]
    bias = np.tile(bias, (2, 1))                             # [128, 3] dup
    blf = np.asarray(b_last, f)[perm]
    blp = np.zeros((128, 4), f)
    blp[:, 0] = blf[0:128]
    blp[:, 1] = blf[128:256]
    blp[0:64, 3] = np.tile(blf[256:288], 2)
    eye = np.tile(np.eye(32, dtype=f), (4, 1))
    out = dict(w0c1=w0c1, w0c2=w0c2, w1P=w1P, w1S=w1S, w2P=w2P, w2S=w2S,
               wlP=wlP, wlS2=wlS2, eye=eye)
    out = {k: v.astype(BF16_NP) for k, v in out.items()}
    out["bias"] = np.ascontiguousarray(bias)
    out["blp"] = blp
    return out


def make_in_maps(z, backbone, mem_stab, mem_unstab, shared):
    f = np.float32
    z = np.asarray(z, f)
    backbone = np.asarray(backbone, f)
    ms = np.asarray(mem_stab, f)
    mu = np.asarray(mem_unstab, f)
    maps = []
    for b in range(z.shape[0]):
        x160 = np.concatenate([backbone[b], z[b], ms[b]], axis=0)
        msf = _pad_rows(ms[b], NCOL)
        muf = _pad_rows(mu[b], MUCOL)
        mu3 = np.concatenate([muf[:, ST * k:ST * k + MUCOL - 2 * ST - 2]
                              for k in range(3)], axis=0)
        mu3 = np.ascontiguousarray(
            np.pad(mu3, ((0, 0), (0, MUCOL - mu3.shape[1]))))
        maps.append(dict(xpad=_pad_rows(x160, NCOL),
                         mu3=mu3,
                         msa=_shift_stack(msf, P_TAPS[0:4]),
                         msb=_shift_stack(msf, P_TAPS[4:8]),
                         ms8=_shift_stack(msf, P_TAPS[8:9]),
                         **shared))
    return maps


_NC_CACHE = {}


def _get_nc(debug=False):
    if debug not in _NC_CACHE:
        _NC_CACHE[debug] = _build_program(debug)
    return _NC_CACHE[debug]


def kernel(z, backbone, mem_stab, mem_unstab, w0, b0, w1, b1, w2, b2,
           w_last, b_last, fusion_kernel_size):
    assert int(fusion_kernel_size) == 3
    shared = _prep_shared(w0, b0, w1, b1, w2, b2, w_last, b_last)
    in_maps = make_in_maps(z, backbone, mem_stab, mem_unstab, shared)
    nc = _get_nc()
    res = run_bass_kernel_spmd(nc, in_maps, core_ids=list(range(len(in_maps))))
    out = np.stack([r["out"] for r in res.results], axis=0)
    return out.astype(np.float32)


# revision 18
# speedup vs baseline: 1.8730x; 1.2567x over previous
"""Trainium2 Bass kernel for nn_ControlledConvEMAStabilizer.

Pipeline (per batch image, one NeuronCore each, batch-parallel over 8 cores):
  q = cat(backbone, z, mem_stab, mem_unstab)          # 160ch
  q = lrelu(conv3x3(q, w0) + b0)                      # -> 64ch
  q = lrelu(conv3x3(q, w1) + b1)                      # -> 64ch
  q = lrelu(conv3x3(q, w2) + b2)                      # -> 64ch
  head = conv3x3(q, w_last) + b_last                  # -> 288ch = 9 taps x 32ch
  eta  = softmax([head; 0]) over the 9+1 slots
  out  = sum_p unfold(mem_stab)[p] * eta[p] + eta[9] * z

Implementation notes:
  - Feature maps live in SBUF as zero-padded flat rows: image pixel (r,c) at
    column 129*(r+1)+1+c (row stride 129, shared single pad column between
    rows, one pad row top/bottom).  Every 3x3 tap is a pure column offset,
    so convs are PSUM-accumulated matmuls over shifted views.
  - K-stacking: intermediates stored twice in one [128, NCOL] tile:
    partitions 0:64 = q, partitions 64:128 = q shifted +129 (one image row).
    A K=128 matmul applies two vertical taps at once.
  - PE sub-array packing via tile_position: strips processed in PAIRS.
    M=64 convs (conv0/1/2) run both strips' matmuls concurrently in the two
    column halves of the PE array (out partitions 0:64 / 64:128).  conv_last
    K=64 tap matmuls row-pair across strips (rows 0:64 / 64:128); the M=32
    head chunk and the softmax-reduction matmuls pack 2- and 4-wide into
    32-column groups.  Measured ~1.8-4x PE throughput vs serial.
  - LeakyReLU evac: single ScalarE activation (Lrelu, alpha=0.01, bias) from
    PSUM into q's strided pixel cells; K-stack upcopy via gpsimd-issued DMA.
  - Fusion tail: exp on ACT, eta*patch products on DVE against host-prepared
    pre-shifted mem_stab tap stacks resident in SBUF (no per-strip DMA),
    partition-group sums via 4-wide packed identity matmuls, recip+mul DVE.
"""

import numpy as np
from contextlib import ExitStack

import concourse.bacc as bacc
import concourse.tile as tile
from concourse import mybir
from concourse.bass_utils import run_bass_kernel_spmd

F32 = mybir.dt.float32
BF16 = mybir.dt.bfloat16
ALU = mybir.AluOpType
ACTF = mybir.ActivationFunctionType

H = 128
ST = 129                      # padded row stride
NCOL = ST * 130 + 2           # 16772 sbuf cols
MUCOL = NCOL + 2 * ST + 2
RPS = 3                       # rows per strip

# taps in fusion/unfold order p = 3*kh + kw -> offset 129*(kh-1) + (kw-1)
P_TAPS = [ST * (kh - 1) + (kw - 1) for kh in range(3) for kw in range(3)]


def _j0(r0):
    return ST * (r0 + 1) + 1


def _strips():
    out, r0 = [], 0
    while r0 < H:
        nr = min(RPS, H - r0)
        out.append((r0, nr))
        r0 += nr
    return out


def _pairs():
    s = _strips()
    out, i = [], 0
    while i < len(s):
        if i + 1 < len(s) and s[i + 1][1] == RPS:
            out.append((s[i], s[i + 1]))
            i += 2
        else:
            out.append((s[i],))
            i += 1
    return out


def _build_program(debug=False):
    nc = bacc.Bacc("TRN2", target_bir_lowering=False, debug=False)

    d_xpad = nc.dram_tensor("xpad", [128, NCOL], BF16, kind="ExternalInput")
    d_mu3 = nc.dram_tensor("mu3", [96, MUCOL], BF16, kind="ExternalInput")
    d_msa = nc.dram_tensor("msa", [128, NCOL], BF16, kind="ExternalInput")
    d_msb = nc.dram_tensor("msb", [128, NCOL], BF16, kind="ExternalInput")
    d_ms8 = nc.dram_tensor("ms8", [32, NCOL], BF16, kind="ExternalInput")
    d_w0c1 = nc.dram_tensor("w0c1", [128, 9 * 64], BF16, kind="ExternalInput")
    d_w0c2 = nc.dram_tensor("w0c2", [96, 3 * 64], BF16, kind="ExternalInput")
    d_w1P = nc.dram_tensor("w1P", [128, 3 * 64], BF16, kind="ExternalInput")
    d_w1S = nc.dram_tensor("w1S", [64, 3 * 64], BF16, kind="ExternalInput")
    d_w2P = nc.dram_tensor("w2P", [128, 3 * 64], BF16, kind="ExternalInput")
    d_w2S = nc.dram_tensor("w2S", [64, 3 * 64], BF16, kind="ExternalInput")
    d_wlP = nc.dram_tensor("wlP", [128, 3 * 288], BF16, kind="ExternalInput")
    d_wlS2 = nc.dram_tensor("wlS2", [128, 3 * 288], BF16, kind="ExternalInput")
    d_b = nc.dram_tensor("bias", [128, 3], F32, kind="ExternalInput")
    d_blp = nc.dram_tensor("blp", [128, 4], F32, kind="ExternalInput")
    d_eye = nc.dram_tensor("eye", [128, 32], BF16, kind="ExternalInput")
    d_out = nc.dram_tensor("out", [32, H, H], F32, kind="ExternalOutput")
    if debug:
        d_q1 = nc.dram_tensor("dbg_q1", [128, NCOL], F32, kind="ExternalOutput")
        d_q2 = nc.dram_tensor("dbg_q2", [128, NCOL], F32, kind="ExternalOutput")
        d_q3 = nc.dram_tensor("dbg_q3", [128, NCOL], F32, kind="ExternalOutput")

    pairs = _pairs()
    strips = _strips()

    with tile.TileContext(nc) as tc, ExitStack() as ctx:
        wp = ctx.enter_context(tc.tile_pool(name="wp", bufs=1))
        big = ctx.enter_context(tc.tile_pool(name="big", bufs=1))
        xs = ctx.enter_context(tc.tile_pool(name="xs", bufs=2))
        fu = ctx.enter_context(tc.tile_pool(name="fu", bufs=2))
        f1 = ctx.enter_context(tc.tile_pool(name="f1", bufs=1))
        pm = ctx.enter_context(tc.tile_pool(name="pm", bufs=2, space="PSUM"))
        pA = ctx.enter_context(tc.tile_pool(name="pA", bufs=2, space="PSUM"))
        pB = ctx.enter_context(tc.tile_pool(name="pB", bufs=2, space="PSUM"))
        pD = ctx.enter_context(tc.tile_pool(name="pD", bufs=2, space="PSUM"))

        # ---- weights / constants / resident stacks to SBUF ----
        w0c1 = wp.tile([128, 9 * 64], BF16)
        w0c2 = wp.tile([96, 3 * 64], BF16)
        w1P = wp.tile([128, 3 * 64], BF16)
        w1S = wp.tile([64, 3 * 64], BF16)
        w2P = wp.tile([128, 3 * 64], BF16)
        w2S = wp.tile([64, 3 * 64], BF16)
        wlP = wp.tile([128, 3 * 288], BF16)
        wlS2 = wp.tile([128, 3 * 288], BF16)
        bias = wp.tile([128, 3], F32)
        blp = wp.tile([128, 4], F32)
        eye = wp.tile([128, 32], BF16)
        msa = wp.tile([128, NCOL], BF16)
        msb = wp.tile([128, NCOL], BF16)
        ms8 = wp.tile([32, NCOL], BF16)
        for dst, src in ((w0c1, d_w0c1), (w0c2, d_w0c2), (w1P, d_w1P),
                         (w1S, d_w1S), (w2P, d_w2P), (w2S, d_w2S),
                         (wlP, d_wlP), (wlS2, d_wlS2), (eye, d_eye),
                         (bias, d_b), (blp, d_blp)):
            nc.sync.dma_start(out=dst[:], in_=src.ap())

        def wsl(wt, i, m0, mw, step=64):
            return wt[:, i * step + m0: i * step + m0 + mw]

        def new_q(tag):
            q = big.tile([128, NCOL], BF16, tag=tag)
            # zero the pad structure (lower half: head, inter-row cells, tail;
            # upper half: head cell + tail region never covered by upcopies)
            nc.gpsimd.memset(q[0:64, 0:130], 0.0)
            inter = q[0:64, 258:258 + 127 * ST].rearrange(
                "p (m s) -> p m s", s=ST)[:, :, 0:1]
            nc.gpsimd.memset(inter, 0.0)
            nc.gpsimd.memset(q[0:64, ST * 129:NCOL], 0.0)
            last_j0, last_n = _j0(strips[-1][0]), strips[-1][1] * ST
            nc.gpsimd.memset(q[64:128, 0:1], 0.0)
            nc.gpsimd.memset(q[64:128, last_j0 - ST + last_n:NCOL], 0.0)
            return q

        def evac(ps, q, pr, bcol, up_sync=False):
            # ps[64*i : 64*i+64] holds strip i's 64ch: leaky-relu contiguous
            # into q (strip 0 on ACT, strip 1 on DVE to balance engines),
            # re-zero the 3 in-strip pad cells, then K-stack upcopy DMA.
            for i, (r0, nr) in enumerate(pr):
                j0, n = _j0(r0), ST * nr
                if i == 0:
                    nc.scalar.activation(q[0:64, j0:j0 + n], ps[0:64, 0:n],
                                         ACTF.Lrelu,
                                         bias=bias[0:64, bcol:bcol + 1],
                                         alpha=0.01)
                else:
                    tmp = fu.tile([64, 3 * ST], F32, tag="lrtmp")
                    nc.vector.tensor_scalar(tmp[:, 0:n], ps[64:128, 0:n],
                                            bias[64:128, bcol:bcol + 1], None,
                                            ALU.add)
                    nc.vector.scalar_tensor_tensor(q[0:64, j0:j0 + n],
                                                   tmp[:, 0:n], 0.01,
                                                   tmp[:, 0:n],
                                                   op0=ALU.mult, op1=ALU.max)
                pv = q[0:64, j0 + 128:j0 + 128 + nr * ST].rearrange(
                    "p (m s) -> p m s", s=ST)[:, :, 0:1]
                nc.gpsimd.memset(pv, 0.0)
                eng = nc.sync if up_sync else nc.gpsimd
                eng.dma_start(out=q[64:128, j0 - ST:j0 - ST + n],
                              in_=q[0:64, j0:j0 + n])

        # ================= conv0 (streamed input, strip-pair groups) ======
        q1 = new_q("A")
        for pr in pairs:
            r0g = pr[0][0]
            nrg = sum(nr for _, nr in pr)
            jg = _j0(r0g)
            win = ST * nrg + 260
            x1 = xs.tile([128, ST * 6 + 260], BF16, tag="x1")
            x2 = xs.tile([96, ST * 6 + 260], BF16, tag="x2")
            nc.sync.dma_start(out=x1[:, 0:win],
                              in_=d_xpad.ap()[:, jg - 130:jg - 130 + win])
            nc.sync.dma_start(out=x2[:, 0:win],
                              in_=d_mu3.ap()[:, jg - 130:jg - 130 + win])
            ps = pm.tile([128, 3 * ST], F32, tag="pm")
            off = [(a, b) for a in (-1, 0, 1) for b in (-1, 0, 1)]
            for t, (dr, dc) in enumerate(off):
                for i, (r0, nr) in enumerate(pr):
                    loc = _j0(r0) - jg + 130
                    o = loc + ST * dr + dc
                    nc.tensor.matmul(ps[64 * i:64 * i + 64, 0:ST * nr],
                                     wsl(w0c1, t, 0, 64), x1[:, o:o + ST * nr],
                                     start=(t == 0), stop=False)
            for t, dc in enumerate((-1, 0, 1)):
                for i, (r0, nr) in enumerate(pr):
                    loc = _j0(r0) - jg + 130
                    o = loc - ST + dc
                    nc.tensor.matmul(ps[64 * i:64 * i + 64, 0:ST * nr],
                                     wsl(w0c2, t, 0, 64), x2[:, o:o + ST * nr],
                                     start=False, stop=(t == 2))
            evac(ps, q1, pr, 0)
        # resident mem_stab tap stacks: issue now so the big transfers ride
        # the idle DMA window during conv1/conv2 (no input staging there)
        for dst, src in ((msa, d_msa), (msb, d_msb), (ms8, d_ms8)):
            nc.gpsimd.dma_start(out=dst[:], in_=src.ap())
        if debug:
            nc.sync.dma_start(out=d_q1.ap(), in_=q1[:])

        # ================= conv1 / conv2 =================
        def mid_conv(qin, qout, wP, wS, bcol):
            for pr in pairs:
                ps = pm.tile([128, 3 * ST], F32, tag="pm")
                for t, dc in enumerate((-1, 0, 1)):
                    for i, (r0, nr) in enumerate(pr):
                        o = _j0(r0) - ST + dc
                        nc.tensor.matmul(ps[64 * i:64 * i + 64, 0:ST * nr],
                                         wsl(wP, t, 0, 64), qin[0:128, o:o + ST * nr],
                                         start=(t == 0), stop=False)
                for t, dc in enumerate((-1, 0, 1)):
                    for i, (r0, nr) in enumerate(pr):
                        o = _j0(r0) + ST + dc
                        nc.tensor.matmul(ps[64 * i:64 * i + 64, 0:ST * nr],
                                         wsl(wS, t, 0, 64), qin[0:64, o:o + ST * nr],
                                         start=False, stop=(t == 2))
                evac(ps, qout, pr, bcol, up_sync=True)

        q2 = new_q("B")
        mid_conv(q1, q2, w1P, w1S, 1)
        if debug:
            nc.sync.dma_start(out=d_q2.ap(), in_=q2[:])
        q3 = new_q("A")
        mid_conv(q2, q3, w2P, w2S, 2)
        if debug:
            nc.sync.dma_start(out=d_q3.ap(), in_=q3[:])

        # ================= conv_last + softmax + fusion =================
        # software-pipelined one pair deep: pair k's reduction matmuls are
        # emitted after pair k+1's conv_last matmuls, so the PE queue never
        # stalls on the exp->mul chain.
        def tail_compute(pr):
            np_ = len(pr)
            j0s = [_j0(r0) for r0, _ in pr]
            ns = [ST * nr for _, nr in pr]
            # z loads (no deps -> issue early)
            rz = [fu.tile([64, 3 * ST], BF16, tag=f"rz{i}", name=f"rz{i}")
                  for i in range(np_)]
            for i in range(np_):
                nc.sync.dma_start(out=rz[i][32:64, 0:ns[i]],
                                  in_=d_xpad.ap()[64:96, j0s[i]:j0s[i] + ns[i]])
            ppool = (pA, pB)
            ea, eb, ec = [], [], []
            # chunk 0 (head channels 0:128) then chunk 1 (128:256)
            for ck, (m0, edst) in enumerate(((0, ea), (128, eb))):
                ph = [ppool[i].tile([128, 3 * ST], F32, tag=f"p{'AB'[i]}",
                                    name=f"ph{i}")
                      for i in range(np_)]
                for t, dc in enumerate((-1, 0, 1)):
                    for i in range(np_):
                        o = j0s[i] - ST + dc
                        nc.tensor.matmul(ph[i][:, 0:ns[i]],
                                         wsl(wlP, t, m0, 128, 288),
                                         q3[0:128, o:o + ns[i]],
                                         start=(t == 0), stop=False)
                for t, dc in enumerate((-1, 0, 1)):
                    # row-paired K=64 taps: strip0 rows 0:64, strip1 64:128
                    for i in range(np_):
                        if i == 0:
                            lhs = wsl(wlS2, t, m0, 128, 288)[0:64]
                            rhs = q3[0:64, j0s[0] + ST + dc:j0s[0] + ST + dc + ns[0]]
                        else:
                            lhs = wsl(wlS2, t, m0, 128, 288)[64:128]
                            rhs = q3[64:128, j0s[1] + dc:j0s[1] + dc + ns[1]]
                        nc.tensor.matmul(ph[i][:, 0:ns[i]], lhs, rhs,
                                         start=False, stop=(t == 2))
                for i in range(np_):
                    e = fu.tile([128, 3 * ST], BF16, tag=f"e{ck}{i}")
                    nc.scalar.activation(e[:, 0:ns[i]], ph[i][:, 0:ns[i]],
                                         ACTF.Exp, bias=blp[:, ck:ck + 1])
                    edst.append(e)
            # chunk 2 (M=32, both strips col-packed into one [64,.] psum);
            # borrows the pm pool (idle in the tail phase) for bufs=2
            phc = pm.tile([128, 3 * ST], F32, tag="pm")
            for t, dc in enumerate((-1, 0, 1)):
                for i in range(np_):
                    o = j0s[i] - ST + dc
                    nc.tensor.matmul(phc[32 * i:32 * i + 32, 0:ns[i]],
                                     wsl(wlP, t, 256, 32, 288),
                                     q3[0:128, o:o + ns[i]],
                                     start=(t == 0), stop=False)
            for t, dc in enumerate((-1, 0, 1)):
                for i in range(np_):
                    o = j0s[i] + ST + dc
                    nc.tensor.matmul(phc[32 * i:32 * i + 32, 0:ns[i]],
                                     wsl(wlS2, t, 256, 32, 288)[0:64],
                                     q3[0:64, o:o + ns[i]],
                                     start=False, stop=(t == 2))
            for i in range(np_):
                e = fu.tile([32, 3 * ST], BF16, tag=f"ec{i}")
                nc.scalar.activation(e[:, 0:ns[i]],
                                     phc[32 * i:32 * i + 32, 0:ns[i]],
                                     ACTF.Exp, bias=blp[32 * i:32 * i + 32, 3:4])
                ec.append(e)
            # eta * patch products (DVE, same-base operands)
            ta, tb = [], []
            for i in range(np_):
                t1 = fu.tile([128, 3 * ST], BF16, tag=f"ta{i}")
                t2 = fu.tile([128, 3 * ST], BF16, tag=f"tb{i}")
                nc.vector.tensor_mul(t1[:, 0:ns[i]], ea[i][:, 0:ns[i]],
                                     msa[:, j0s[i]:j0s[i] + ns[i]])
                nc.vector.tensor_mul(t2[:, 0:ns[i]], eb[i][:, 0:ns[i]],
                                     msb[:, j0s[i]:j0s[i] + ns[i]])
                nc.vector.tensor_mul(rz[i][0:32, 0:ns[i]], ec[i][:, 0:ns[i]],
                                     ms8[:, j0s[i]:j0s[i] + ns[i]])
                ta.append(t1)
                tb.append(t2)
            return (pr, j0s, ns, rz, ea, eb, ec, ta, tb)

        def tail_fusion(state):
            pr, j0s, ns, rz, ea, eb, ec, ta, tb = state
            np_ = len(pr)
            # packed reduction matmuls: num strip i -> nd[32i:32i+32],
            # den strip i -> nd[64+32i : 96+32i]
            nd = pD.tile([128, 3 * ST], F32, tag="pD")
            for t in range(3):
                for i in range(np_):
                    npos = 32 * i
                    dpos = 64 + 32 * i
                    nl, nr_ = ((eye[:], ta[i]), (eye[:], tb[i]),
                               (eye[0:64], rz[i]))[t]
                    dl, dr = ((eye[:], ea[i]), (eye[:], eb[i]),
                              (eye[0:32], ec[i]))[t]
                    nc.tensor.matmul(nd[npos:npos + 32, 0:ns[i]], nl,
                                     nr_[:, 0:ns[i]], start=(t == 0),
                                     stop=(t == 2), tile_position=(0, npos))
                    nc.tensor.matmul(nd[dpos:dpos + 32, 0:ns[i]], dl,
                                     dr[:, 0:ns[i]], start=(t == 0),
                                     stop=(t == 2), tile_position=(0, dpos))
            # rde = 1/(den+1); ost = num * rde  (both strips at once)
            w = 32 * np_
            den = f1.tile([64, 3 * ST], F32, tag="den")
            rde = f1.tile([64, 3 * ST], F32, tag="rde")
            ost = f1.tile([64, 3 * ST], F32, tag="ost")
            nmax = max(ns)
            nc.vector.tensor_scalar_add(den[0:w, 0:nmax],
                                        nd[64:64 + w, 0:nmax], 1.0)
            nc.vector.reciprocal_approx_fast(rde[0:w, 0:nmax], den[0:w, 0:nmax])
            nc.vector.tensor_mul(ost[0:w, 0:nmax], nd[0:w, 0:nmax],
                                 rde[0:w, 0:nmax])
            for i, (r0, nr) in enumerate(pr):
                src = ost[32 * i:32 * i + 32, 0:ns[i]].rearrange(
                    "p (r c) -> p r c", c=ST)[:, :, 0:128]
                nc.sync.dma_start(out=d_out.ap()[:, r0:r0 + nr, :], in_=src)

        prev = None
        for pr in pairs:
            st = tail_compute(pr)
            if prev is not None:
                tail_fusion(prev)
            prev = st
        tail_fusion(prev)

    nc.compile()
    return nc


BF16_NP = mybir.dt.np(mybir.dt.bfloat16)


def _pad_rows(x, cols):
    # x: [C, 128, 128] -> zero-padded flat rows [C, cols], bf16
    c = x.shape[0]
    buf = np.zeros((c, cols), dtype=BF16_NP)
    buf[:, 130:130 + ST * 128].reshape(c, 128, ST)[:, :, 0:128] = x.astype(BF16_NP)
    return buf


def _shift_stack(flat, offs):
    # flat: [32, NCOL]; returns [32*len(offs), NCOL] rows shifted by offs
    ext = np.zeros((flat.shape[0], NCOL + 264), dtype=flat.dtype)
    ext[:, 132:132 + NCOL] = flat
    return np.concatenate([ext[:, 132 + o:132 + o + NCOL] for o in offs], axis=0)


def _prep_shared(w0, b0, w1, b1, w2, b2, w_last, b_last):
    f = np.float32
    w0t = np.transpose(np.asarray(w0, f), (1, 2, 3, 0))      # [160,3,3,64]
    w0c1 = np.ascontiguousarray(w0t[0:128].reshape(128, 9 * 64))
    w0c2 = np.ascontiguousarray(
        np.transpose(w0t[128:160], (1, 0, 2, 3)).reshape(96, 3 * 64))

    def mid(w):
        wt = np.transpose(np.asarray(w, f), (1, 2, 3, 0))    # [64,3,3,64]
        wP = np.ascontiguousarray(
            np.concatenate([wt[:, 0], wt[:, 1]], 0).reshape(128, 3 * 64))
        wS = np.ascontiguousarray(wt[:, 2].reshape(64, 3 * 64))
        return wP, wS

    w1P, w1S = mid(w1)
    w2P, w2S = mid(w2)
    perm = np.array([(pp % 32) * 9 + pp // 32 for pp in range(288)])
    wl2 = np.asarray(w_last, f)[perm]                        # [288,64,3,3]
    wlt = np.transpose(wl2, (1, 2, 3, 0))                    # [64,3,3,288]
    wlP = np.ascontiguousarray(
        np.concatenate([wlt[:, 0], wlt[:, 1]], 0).reshape(128, 3 * 288))
    wlS = np.ascontiguousarray(wlt[:, 2].reshape(64, 3 * 288))
    wlS2 = np.concatenate([wlS, wlS], axis=0)                # [128, 864]
    bias = np.stack([np.asarray(b0, f), np.asarray(b1, f),
                     np.asarray(b2, f)], axis=1)             # [64, 3]
    bias = np.tile(bias, (2, 1))                             # [128, 3] dup
    blf = np.asarray(b_last, f)[perm]
    blp = np.zeros((128, 4), f)
    blp[:, 0] = blf[0:128]
    blp[:, 1] = blf[128:256]
    blp[0:64, 3] = np.tile(blf[256:288], 2)
    eye = np.tile(np.eye(32, dtype=f), (4, 1))
    out = dict(w0c1=w0c1, w0c2=w0c2, w1P=w1P, w1S=w1S, w2P=w2P, w2S=w2S,
               wlP=wlP, wlS2=wlS2, eye=eye)
    out = {k: v.astype(BF16_NP) for k, v in out.items()}
    out["bias"] = np.ascontiguousarray(bias)
    out["blp"] = blp
    return out


def make_in_maps(z, backbone, mem_stab, mem_unstab, shared):
    f = np.float32
    z = np.asarray(z, f)
    backbone = np.asarray(backbone, f)
    ms = np.asarray(mem_stab, f)
    mu = np.asarray(mem_unstab, f)
    maps = []
    for b in range(z.shape[0]):
        x160 = np.concatenate([backbone[b], z[b], ms[b]], axis=0)
        msf = _pad_rows(ms[b], NCOL)
        muf = _pad_rows(mu[b], MUCOL)
        mu3 = np.concatenate([muf[:, ST * k:ST * k + MUCOL - 2 * ST - 2]
                              for k in range(3)], axis=0)
        mu3 = np.ascontiguousarray(
            np.pad(mu3, ((0, 0), (0, MUCOL - mu3.shape[1]))))
        maps.append(dict(xpad=_pad_rows(x160, NCOL),
                         mu3=mu3,
                         msa=_shift_stack(msf, P_TAPS[0:4]),
                         msb=_shift_stack(msf, P_TAPS[4:8]),
                         ms8=_shift_stack(msf, P_TAPS[8:9]),
                         **shared))
    return maps


_NC_CACHE = {}


def _get_nc(debug=False):
    if debug not in _NC_CACHE:
        _NC_CACHE[debug] = _build_program(debug)
    return _NC_CACHE[debug]


def kernel(z, backbone, mem_stab, mem_unstab, w0, b0, w1, b1, w2, b2,
           w_last, b_last, fusion_kernel_size):
    assert int(fusion_kernel_size) == 3
    shared = _prep_shared(w0, b0, w1, b1, w2, b2, w_last, b_last)
    in_maps = make_in_maps(z, backbone, mem_stab, mem_unstab, shared)
    nc = _get_nc()
    res = run_bass_kernel_spmd(nc, in_maps, core_ids=list(range(len(in_maps))))
    out = np.stack([r["out"] for r in res.results], axis=0)
    return out.astype(np.float32)


# revision 22
# speedup vs baseline: 1.8755x; 1.0014x over previous
"""Trainium2 Bass kernel for nn_ControlledConvEMAStabilizer.

Pipeline (per batch image, one NeuronCore each, batch-parallel over 8 cores):
  q = cat(backbone, z, mem_stab, mem_unstab)          # 160ch
  q = lrelu(conv3x3(q, w0) + b0)                      # -> 64ch
  q = lrelu(conv3x3(q, w1) + b1)                      # -> 64ch
  q = lrelu(conv3x3(q, w2) + b2)                      # -> 64ch
  head = conv3x3(q, w_last) + b_last                  # -> 288ch = 9 taps x 32ch
  eta  = softmax([head; 0]) over the 9+1 slots
  out  = sum_p unfold(mem_stab)[p] * eta[p] + eta[9] * z

Implementation notes:
  - Feature maps live in SBUF as zero-padded flat rows: image pixel (r,c) at
    column 129*(r+1)+1+c (row stride 129, shared single pad column between
    rows, one pad row top/bottom).  Every 3x3 tap is a pure column offset,
    so convs are PSUM-accumulated matmuls over shifted views.
  - K-stacking: intermediates stored twice in one [128, NCOL] tile:
    partitions 0:64 = q, partitions 64:128 = q shifted +129 (one image row).
    A K=128 matmul applies two vertical taps at once.
  - PE sub-array packing via tile_position: strips processed in PAIRS.
    M=64 convs (conv0/1/2) run both strips' matmuls concurrently in the two
    column halves of the PE array (out partitions 0:64 / 64:128).  conv_last
    K=64 tap matmuls row-pair across strips (rows 0:64 / 64:128); the M=32
    head chunk and the softmax-reduction matmuls pack 2- and 4-wide into
    32-column groups.  Measured ~1.8-4x PE throughput vs serial.
  - LeakyReLU evac: single ScalarE activation (Lrelu, alpha=0.01, bias) from
    PSUM into q's strided pixel cells; K-stack upcopy via gpsimd-issued DMA.
  - Fusion tail: exp on ACT, eta*patch products on DVE against host-prepared
    pre-shifted mem_stab tap stacks resident in SBUF (no per-strip DMA),
    partition-group sums via 4-wide packed identity matmuls, recip+mul DVE.
"""

import numpy as np
from contextlib import ExitStack

import concourse.bacc as bacc
import concourse.tile as tile
from concourse import mybir
from concourse.bass_utils import run_bass_kernel_spmd

F32 = mybir.dt.float32
BF16 = mybir.dt.bfloat16
ALU = mybir.AluOpType
ACTF = mybir.ActivationFunctionType

H = 128
ST = 129                      # padded row stride
NCOL = ST * 130 + 2           # 16772 sbuf cols
MUCOL = NCOL + 2 * ST + 2
RPS = 3                       # rows per strip

# taps in fusion/unfold order p = 3*kh + kw -> offset 129*(kh-1) + (kw-1)
P_TAPS = [ST * (kh - 1) + (kw - 1) for kh in range(3) for kw in range(3)]


def _j0(r0):
    return ST * (r0 + 1) + 1


def _strips():
    out, r0 = [], 0
    while r0 < H:
        nr = min(RPS, H - r0)
        out.append((r0, nr))
        r0 += nr
    return out


def _pairs():
    s = _strips()
    out, i = [], 0
    while i < len(s):
        if i + 1 < len(s) and s[i + 1][1] == RPS:
            out.append((s[i], s[i + 1]))
            i += 2
        else:
            out.append((s[i],))
            i += 1
    return out


def _build_program(debug=False):
    nc = bacc.Bacc("TRN2", target_bir_lowering=False, debug=False)

    d_xpad = nc.dram_tensor("xpad", [128, NCOL], BF16, kind="ExternalInput")
    d_mu3 = nc.dram_tensor("mu3", [96, MUCOL], BF16, kind="ExternalInput")
    d_msa = nc.dram_tensor("msa", [128, NCOL], BF16, kind="ExternalInput")
    d_msb = nc.dram_tensor("msb", [128, NCOL], BF16, kind="ExternalInput")
    d_ms8 = nc.dram_tensor("ms8", [32, NCOL], BF16, kind="ExternalInput")
    d_w0c1 = nc.dram_tensor("w0c1", [128, 9 * 64], BF16, kind="ExternalInput")
    d_w0c2 = nc.dram_tensor("w0c2", [96, 3 * 64], BF16, kind="ExternalInput")
    d_w1P = nc.dram_tensor("w1P", [128, 3 * 64], BF16, kind="ExternalInput")
    d_w1S = nc.dram_tensor("w1S", [64, 3 * 64], BF16, kind="ExternalInput")
    d_w2P = nc.dram_tensor("w2P", [128, 3 * 64], BF16, kind="ExternalInput")
    d_w2S = nc.dram_tensor("w2S", [64, 3 * 64], BF16, kind="ExternalInput")
    d_wlP = nc.dram_tensor("wlP", [128, 3 * 288], BF16, kind="ExternalInput")
    d_wlS2 = nc.dram_tensor("wlS2", [128, 3 * 288], BF16, kind="ExternalInput")
    d_b = nc.dram_tensor("bias", [128, 3], F32, kind="ExternalInput")
    d_blp = nc.dram_tensor("blp", [128, 4], F32, kind="ExternalInput")
    d_eye = nc.dram_tensor("eye", [128, 32], BF16, kind="ExternalInput")
    d_out = nc.dram_tensor("out", [32, H, H], F32, kind="ExternalOutput")
    if debug:
        d_q1 = nc.dram_tensor("dbg_q1", [128, NCOL], F32, kind="ExternalOutput")
        d_q2 = nc.dram_tensor("dbg_q2", [128, NCOL], F32, kind="ExternalOutput")
        d_q3 = nc.dram_tensor("dbg_q3", [128, NCOL], F32, kind="ExternalOutput")

    pairs = _pairs()
    strips = _strips()

    with tile.TileContext(nc) as tc, ExitStack() as ctx:
        wp = ctx.enter_context(tc.tile_pool(name="wp", bufs=1))
        big = ctx.enter_context(tc.tile_pool(name="big", bufs=1))
        xs = ctx.enter_context(tc.tile_pool(name="xs", bufs=2))
        fu = ctx.enter_context(tc.tile_pool(name="fu", bufs=2))
        f1 = ctx.enter_context(tc.tile_pool(name="f1", bufs=1))
        pm = ctx.enter_context(tc.tile_pool(name="pm", bufs=2, space="PSUM"))
        pA = ctx.enter_context(tc.tile_pool(name="pA", bufs=2, space="PSUM"))
        pB = ctx.enter_context(tc.tile_pool(name="pB", bufs=2, space="PSUM"))
        pD = ctx.enter_context(tc.tile_pool(name="pD", bufs=2, space="PSUM"))

        # ---- weights / constants / resident stacks to SBUF ----
        w0c1 = wp.tile([128, 9 * 64], BF16)
        w0c2 = wp.tile([96, 3 * 64], BF16)
        w1P = wp.tile([128, 3 * 64], BF16)
        w1S = wp.tile([64, 3 * 64], BF16)
        w2P = wp.tile([128, 3 * 64], BF16)
        w2S = wp.tile([64, 3 * 64], BF16)
        wlP = wp.tile([128, 3 * 288], BF16)
        wlS2 = wp.tile([128, 3 * 288], BF16)
        bias = wp.tile([128, 3], F32)
        blp = wp.tile([128, 4], F32)
        eye = wp.tile([128, 32], BF16)
        msa = wp.tile([128, NCOL], BF16)
        msb = wp.tile([128, NCOL], BF16)
        ms8 = wp.tile([32, NCOL], BF16)
        wl_eng = (nc.sync, nc.gpsimd, nc.scalar)
        for k, (dst, src) in enumerate(
                ((w0c1, d_w0c1), (w0c2, d_w0c2), (w1P, d_w1P),
                 (w1S, d_w1S), (w2P, d_w2P), (w2S, d_w2S),
                 (wlP, d_wlP), (wlS2, d_wlS2), (eye, d_eye),
                 (bias, d_b), (blp, d_blp))):
            wl_eng[k % 3].dma_start(out=dst[:], in_=src.ap())

        def wsl(wt, i, m0, mw, step=64):
            return wt[:, i * step + m0: i * step + m0 + mw]

        def new_q(tag):
            q = big.tile([128, NCOL], BF16, tag=tag)
            # zero the pad structure (lower half: head, inter-row cells, tail;
            # upper half: head cell + tail region never covered by upcopies)
            nc.gpsimd.memset(q[0:64, 0:130], 0.0)
            inter = q[0:64, 258:258 + 127 * ST].rearrange(
                "p (m s) -> p m s", s=ST)[:, :, 0:1]
            nc.gpsimd.memset(inter, 0.0)
            nc.gpsimd.memset(q[0:64, ST * 129:NCOL], 0.0)
            last_j0, last_n = _j0(strips[-1][0]), strips[-1][1] * ST
            nc.gpsimd.memset(q[64:128, 0:1], 0.0)
            nc.gpsimd.memset(q[64:128, last_j0 - ST + last_n:NCOL], 0.0)
            return q

        def evac(ps, q, pr, bcol, up_sync=False):
            # ps[64*i : 64*i+64] holds strip i's 64ch: leaky-relu contiguous
            # into q (strip 0 on ACT, strip 1 on DVE to balance engines),
            # re-zero the 3 in-strip pad cells, then K-stack upcopy DMA.
            for i, (r0, nr) in enumerate(pr):
                j0, n = _j0(r0), ST * nr
                if i == 0:
                    nc.scalar.activation(q[0:64, j0:j0 + n], ps[0:64, 0:n],
                                         ACTF.Lrelu,
                                         bias=bias[0:64, bcol:bcol + 1],
                                         alpha=0.01)
                else:
                    tmp = fu.tile([64, 3 * ST], F32, tag="lrtmp")
                    nc.vector.tensor_scalar(tmp[:, 0:n], ps[64:128, 0:n],
                                            bias[64:128, bcol:bcol + 1], None,
                                            ALU.add)
                    nc.vector.scalar_tensor_tensor(q[0:64, j0:j0 + n],
                                                   tmp[:, 0:n], 0.01,
                                                   tmp[:, 0:n],
                                                   op0=ALU.mult, op1=ALU.max)
                pv = q[0:64, j0 + 128:j0 + 128 + nr * ST].rearrange(
                    "p (m s) -> p m s", s=ST)[:, :, 0:1]
                nc.gpsimd.memset(pv, 0.0)
                eng = nc.sync if up_sync else nc.gpsimd
                eng.dma_start(out=q[64:128, j0 - ST:j0 - ST + n],
                              in_=q[0:64, j0:j0 + n])

        # ================= conv0 (streamed input, strip-pair groups) ======
        q1 = new_q("A")
        for pr in pairs:
            r0g = pr[0][0]
            nrg = sum(nr for _, nr in pr)
            jg = _j0(r0g)
            win = ST * nrg + 260
            x1 = xs.tile([128, ST * 6 + 260], BF16, tag="x1")
            x2 = xs.tile([96, ST * 6 + 260], BF16, tag="x2")
            nc.scalar.dma_start(out=x1[:, 0:win],
                                in_=d_xpad.ap()[:, jg - 130:jg - 130 + win])
            nc.sync.dma_start(out=x2[:, 0:win],
                              in_=d_mu3.ap()[:, jg - 130:jg - 130 + win])
            ps = pm.tile([128, 3 * ST], F32, tag="pm")
            off = [(a, b) for a in (-1, 0, 1) for b in (-1, 0, 1)]
            for t, (dr, dc) in enumerate(off):
                for i, (r0, nr) in enumerate(pr):
                    loc = _j0(r0) - jg + 130
                    o = loc + ST * dr + dc
                    nc.tensor.matmul(ps[64 * i:64 * i + 64, 0:ST * nr],
                                     wsl(w0c1, t, 0, 64), x1[:, o:o + ST * nr],
                                     start=(t == 0), stop=False)
            for t, dc in enumerate((-1, 0, 1)):
                for i, (r0, nr) in enumerate(pr):
                    loc = _j0(r0) - jg + 130
                    o = loc - ST + dc
                    nc.tensor.matmul(ps[64 * i:64 * i + 64, 0:ST * nr],
                                     wsl(w0c2, t, 0, 64), x2[:, o:o + ST * nr],
                                     start=False, stop=(t == 2))
            evac(ps, q1, pr, 0)
        # resident mem_stab tap stacks: defer so the big transfers ride the
        # idle DMA window during conv1/conv2 instead of starving conv0's
        # input staging (the scheduler hoists dependency-free DMAs to t=0)
        with tc.tile_wait_until(0.05):
            for dst, src in ((msa, d_msa), (msb, d_msb), (ms8, d_ms8)):
                nc.gpsimd.dma_start(out=dst[:], in_=src.ap())
        if debug:
            nc.sync.dma_start(out=d_q1.ap(), in_=q1[:])

        # ================= conv1 / conv2 =================
        def mid_conv(qin, qout, wP, wS, bcol):
            for pr in pairs:
                ps = pm.tile([128, 3 * ST], F32, tag="pm")
                for t, dc in enumerate((-1, 0, 1)):
                    for i, (r0, nr) in enumerate(pr):
                        o = _j0(r0) - ST + dc
                        nc.tensor.matmul(ps[64 * i:64 * i + 64, 0:ST * nr],
                                         wsl(wP, t, 0, 64), qin[0:128, o:o + ST * nr],
                                         start=(t == 0), stop=False)
                for t, dc in enumerate((-1, 0, 1)):
                    for i, (r0, nr) in enumerate(pr):
                        o = _j0(r0) + ST + dc
                        nc.tensor.matmul(ps[64 * i:64 * i + 64, 0:ST * nr],
                                         wsl(wS, t, 0, 64), qin[0:64, o:o + ST * nr],
                                         start=False, stop=(t == 2))
                evac(ps, qout, pr, bcol, up_sync=True)

        q2 = new_q("B")
        mid_conv(q1, q2, w1P, w1S, 1)
        if debug:
            nc.sync.dma_start(out=d_q2.ap(), in_=q2[:])
        q3 = new_q("A")
        mid_conv(q2, q3, w2P, w2S, 2)
        if debug:
            nc.sync.dma_start(out=d_q3.ap(), in_=q3[:])

        # ================= conv_last + softmax + fusion =================
        # software-pipelined one pair deep: pair k's reduction matmuls are
        # emitted after pair k+1's conv_last matmuls, so the PE queue never
        # stalls on the exp->mul chain.
        def tail_compute(pr):
            np_ = len(pr)
            j0s = [_j0(r0) for r0, _ in pr]
            ns = [ST * nr for _, nr in pr]
            # z loads (no deps -> issue early)
            rz = [fu.tile([64, 3 * ST], BF16, tag=f"rz{i}", name=f"rz{i}")
                  for i in range(np_)]
            for i in range(np_):
                nc.sync.dma_start(out=rz[i][32:64, 0:ns[i]],
                                  in_=d_xpad.ap()[64:96, j0s[i]:j0s[i] + ns[i]])
            ppool = (pA, pB)
            ea, eb, ec = [], [], []
            # chunk 0 (head channels 0:128) then chunk 1 (128:256)
            for ck, (m0, edst) in enumerate(((0, ea), (128, eb))):
                ph = [ppool[i].tile([128, 3 * ST], F32, tag=f"p{'AB'[i]}",
                                    name=f"ph{i}")
                      for i in range(np_)]
                for t, dc in enumerate((-1, 0, 1)):
                    for i in range(np_):
                        o = j0s[i] - ST + dc
                        nc.tensor.matmul(ph[i][:, 0:ns[i]],
                                         wsl(wlP, t, m0, 128, 288),
                                         q3[0:128, o:o + ns[i]],
                                         start=(t == 0), stop=False)
                for t, dc in enumerate((-1, 0, 1)):
                    # row-paired K=64 taps: strip0 rows 0:64, strip1 64:128
                    for i in range(np_):
                        if i == 0:
                            lhs = wsl(wlS2, t, m0, 128, 288)[0:64]
                            rhs = q3[0:64, j0s[0] + ST + dc:j0s[0] + ST + dc + ns[0]]
                        else:
                            lhs = wsl(wlS2, t, m0, 128, 288)[64:128]
                            rhs = q3[64:128, j0s[1] + dc:j0s[1] + dc + ns[1]]
                        nc.tensor.matmul(ph[i][:, 0:ns[i]], lhs, rhs,
                                         start=False, stop=(t == 2))
                for i in range(np_):
                    e = fu.tile([128, 3 * ST], BF16, tag=f"e{ck}{i}")
                    nc.scalar.activation(e[:, 0:ns[i]], ph[i][:, 0:ns[i]],
                                         ACTF.Exp, bias=blp[:, ck:ck + 1])
                    edst.append(e)
            # chunk 2 (M=32, both strips col-packed into one [64,.] psum);
            # borrows the pm pool (idle in the tail phase) for bufs=2
            phc = pm.tile([128, 3 * ST], F32, tag="pm")
            for t, dc in enumerate((-1, 0, 1)):
                for i in range(np_):
                    o = j0s[i] - ST + dc
                    nc.tensor.matmul(phc[32 * i:32 * i + 32, 0:ns[i]],
                                     wsl(wlP, t, 256, 32, 288),
                                     q3[0:128, o:o + ns[i]],
                                     start=(t == 0), stop=False)
            for t, dc in enumerate((-1, 0, 1)):
                for i in range(np_):
                    o = j0s[i] + ST + dc
                    nc.tensor.matmul(phc[32 * i:32 * i + 32, 0:ns[i]],
                                     wsl(wlS2, t, 256, 32, 288)[0:64],
                                     q3[0:64, o:o + ns[i]],
                                     start=False, stop=(t == 2))
            for i in range(np_):
                e = fu.tile([32, 3 * ST], BF16, tag=f"ec{i}")
                nc.scalar.activation(e[:, 0:ns[i]],
                                     phc[32 * i:32 * i + 32, 0:ns[i]],
                                     ACTF.Exp, bias=blp[32 * i:32 * i + 32, 3:4])
                ec.append(e)
            # eta * patch products (DVE, same-base operands)
            ta, tb = [], []
            for i in range(np_):
                t1 = fu.tile([128, 3 * ST], BF16, tag=f"ta{i}")
                t2 = fu.tile([128, 3 * ST], BF16, tag=f"tb{i}")
                nc.vector.tensor_mul(t1[:, 0:ns[i]], ea[i][:, 0:ns[i]],
                                     msa[:, j0s[i]:j0s[i] + ns[i]])
                nc.vector.tensor_mul(t2[:, 0:ns[i]], eb[i][:, 0:ns[i]],
                                     msb[:, j0s[i]:j0s[i] + ns[i]])
                nc.vector.tensor_mul(rz[i][0:32, 0:ns[i]], ec[i][:, 0:ns[i]],
                                     ms8[:, j0s[i]:j0s[i] + ns[i]])
                ta.append(t1)
                tb.append(t2)
            return (pr, j0s, ns, rz, ea, eb, ec, ta, tb)

        def tail_fusion(state):
            pr, j0s, ns, rz, ea, eb, ec, ta, tb = state
            np_ = len(pr)
            # packed reduction matmuls: num strip i -> nd[32i:32i+32],
            # den strip i -> nd[64+32i : 96+32i]
            nd = pD.tile([128, 3 * ST], F32, tag="pD")
            for t in range(3):
                for i in range(np_):
                    npos = 32 * i
                    dpos = 64 + 32 * i
                    nl, nr_ = ((eye[:], ta[i]), (eye[:], tb[i]),
                               (eye[0:64], rz[i]))[t]
                    dl, dr = ((eye[:], ea[i]), (eye[:], eb[i]),
                              (eye[0:32], ec[i]))[t]
                    nc.tensor.matmul(nd[npos:npos + 32, 0:ns[i]], nl,
                                     nr_[:, 0:ns[i]], start=(t == 0),
                                     stop=(t == 2), tile_position=(0, npos))
                    nc.tensor.matmul(nd[dpos:dpos + 32, 0:ns[i]], dl,
                                     dr[:, 0:ns[i]], start=(t == 0),
                                     stop=(t == 2), tile_position=(0, dpos))
            # rde = 1/(den+1); ost = num * rde  (both strips at once)
            w = 32 * np_
            den = f1.tile([64, 3 * ST], F32, tag="den")
            rde = f1.tile([64, 3 * ST], F32, tag="rde")
            ost = f1.tile([64, 3 * ST], F32, tag="ost")
            nmax = max(ns)
            nc.vector.tensor_scalar_add(den[0:w, 0:nmax],
                                        nd[64:64 + w, 0:nmax], 1.0)
            nc.vector.reciprocal_approx_fast(rde[0:w, 0:nmax], den[0:w, 0:nmax])
            nc.vector.tensor_mul(ost[0:w, 0:nmax], nd[0:w, 0:nmax],
                                 rde[0:w, 0:nmax])
            for i, (r0, nr) in enumerate(pr):
                src = ost[32 * i:32 * i + 32, 0:ns[i]].rearrange(
                    "p (r c) -> p r c", c=ST)[:, :, 0:128]
                nc.sync.dma_start(out=d_out.ap()[:, r0:r0 + nr, :], in_=src)

        prev = None
        for pr in pairs:
            st = tail_compute(pr)
            if prev is not None:
                tail_fusion(prev)
            prev = st
        tail_fusion(prev)

    nc.compile()
    return nc


BF16_NP = mybir.dt.np(mybir.dt.bfloat16)


def _pad_rows(x, cols):
    # x: [C, 128, 128] -> zero-padded flat rows [C, cols], bf16
    c = x.shape[0]
    buf = np.zeros((c, cols), dtype=BF16_NP)
    buf[:, 130:130 + ST * 128].reshape(c, 128, ST)[:, :, 0:128] = x.astype(BF16_NP)
    return buf


def _shift_stack(flat, offs):
    # flat: [32, NCOL]; returns [32*len(offs), NCOL] rows shifted by offs
    ext = np.zeros((flat.shape[0], NCOL + 264), dtype=flat.dtype)
    ext[:, 132:132 + NCOL] = flat
    return np.concatenate([ext[:, 132 + o:132 + o + NCOL] for o in offs], axis=0)


def _prep_shared(w0, b0, w1, b1, w2, b2, w_last, b_last):
    f = np.float32
    w0t = np.transpose(np.asarray(w0, f), (1, 2, 3, 0))      # [160,3,3,64]
    w0c1 = np.ascontiguousarray(w0t[0:128].reshape(128, 9 * 64))
    w0c2 = np.ascontiguousarray(
        np.transpose(w0t[128:160], (1, 0, 2, 3)).reshape(96, 3 * 64))

    def mid(w):
        wt = np.transpose(np.asarray(w, f), (1, 2, 3, 0))    # [64,3,3,64]
        wP = np.ascontiguousarray(
            np.concatenate([wt[:, 0], wt[:, 1]], 0).reshape(128, 3 * 64))
        wS = np.ascontiguousarray(wt[:, 2].reshape(64, 3 * 64))
        return wP, wS

    w1P, w1S = mid(w1)
    w2P, w2S = mid(w2)
    perm = np.array([(pp % 32) * 9 + pp // 32 for pp in range(288)])
    wl2 = np.asarray(w_last, f)[perm]                        # [288,64,3,3]
    wlt = np.transpose(wl2, (1, 2, 3, 0))                    # [64,3,3,288]
    wlP = np.ascontiguousarray(
        np.concatenate([wlt[:, 0], wlt[:, 1]], 0).reshape(128, 3 * 288))
    wlS = np.ascontiguousarray(wlt[:, 2].reshape(64, 3 * 288))
    wlS2 = np.concatenate([wlS, wlS], axis=0)                # [128, 864]
    bias = np.stack([np.asarray(b0, f), np.asarray(b1, f),
                     np.asarray(b2, f)], axis=1)             # [64, 3]
    bias = np.tile(bias, (2, 1))                             # [128, 3] dup
    blf = np.asarray(b_last, f)[perm]
    blp = np.zeros((128, 4), f)
    blp[:, 0] = blf[0:128]
    blp[:, 1] = blf[128:256]
    blp[0:64, 3] = np.tile(blf[256:288], 2)
    eye = np.tile(np.eye(32, dtype=f), (4, 1))
    out = dict(w0c1=w0c1, w0c2=w0c2, w1P=w1P, w1S=w1S, w2P=w2P, w2S=w2S,
               wlP=wlP, wlS2=wlS2, eye=eye)
    out = {k: v.astype(BF16_NP) for k, v in out.items()}
    out["bias"] = np.ascontiguousarray(bias)
    out["blp"] = blp
    return out


def make_in_maps(z, backbone, mem_stab, mem_unstab, shared):
    f = np.float32
    z = np.asarray(z, f)
    backbone = np.asarray(backbone, f)
    ms = np.asarray(mem_stab, f)
    mu = np.asarray(mem_unstab, f)
    maps = []
    for b in range(z.shape[0]):
        x160 = np.concatenate([backbone[b], z[b], ms[b]], axis=0)
        msf = _pad_rows(ms[b], NCOL)
        muf = _pad_rows(mu[b], MUCOL)
        mu3 = np.concatenate([muf[:, ST * k:ST * k + MUCOL - 2 * ST - 2]
                              for k in range(3)], axis=0)
        mu3 = np.ascontiguousarray(
            np.pad(mu3, ((0, 0), (0, MUCOL - mu3.shape[1]))))
        maps.append(dict(xpad=_pad_rows(x160, NCOL),
                         mu3=mu3,
                         msa=_shift_stack(msf, P_TAPS[0:4]),
                         msb=_shift_stack(msf, P_TAPS[4:8]),
                         ms8=_shift_stack(msf, P_TAPS[8:9]),
                         **shared))
    return maps


_NC_CACHE = {}


def _get_nc(debug=False):
    if debug not in _NC_CACHE:
        _NC_CACHE[debug] = _build_program(debug)
    return _NC_CACHE[debug]


def kernel(z, backbone, mem_stab, mem_unstab, w0, b0, w1, b1, w2, b2,
           w_last, b_last, fusion_kernel_size):
    assert int(fusion_kernel_size) == 3
    shared = _prep_shared(w0, b0, w1, b1, w2, b2, w_last, b_last)
    in_maps = make_in_maps(z, backbone, mem_stab, mem_unstab, shared)
    nc = _get_nc()
    res = run_bass_kernel_spmd(nc, in_maps, core_ids=list(range(len(in_maps))))
    out = np.stack([r["out"] for r in res.results], axis=0)
    return out.astype(np.float32)


# revision 23
# speedup vs baseline: 1.9276x; 1.0277x over previous
"""Trainium2 Bass kernel for nn_ControlledConvEMAStabilizer.

Pipeline (per batch image, one NeuronCore each, batch-parallel over 8 cores):
  q = cat(backbone, z, mem_stab, mem_unstab)          # 160ch
  q = lrelu(conv3x3(q, w0) + b0)                      # -> 64ch
  q = lrelu(conv3x3(q, w1) + b1)                      # -> 64ch
  q = lrelu(conv3x3(q, w2) + b2)                      # -> 64ch
  head = conv3x3(q, w_last) + b_last                  # -> 288ch = 9 taps x 32ch
  eta  = softmax([head; 0]) over the 9+1 slots
  out  = sum_p unfold(mem_stab)[p] * eta[p] + eta[9] * z

Implementation notes:
  - Feature maps live in SBUF as zero-padded flat rows: image pixel (r,c) at
    column 129*(r+1)+1+c (row stride 129, shared single pad column between
    rows, one pad row top/bottom).  Every 3x3 tap is a pure column offset,
    so convs are PSUM-accumulated matmuls over shifted views.
  - K-stacking: intermediates stored twice in one [128, NCOL] tile:
    partitions 0:64 = q, partitions 64:128 = q shifted +129 (one image row).
    A K=128 matmul applies two vertical taps at once.
  - PE sub-array packing via tile_position: strips processed in PAIRS.
    M=64 convs (conv0/1/2) run both strips' matmuls concurrently in the two
    column halves of the PE array (out partitions 0:64 / 64:128).  conv_last
    K=64 tap matmuls row-pair across strips (rows 0:64 / 64:128); the M=32
    head chunk and the softmax-reduction matmuls pack 2- and 4-wide into
    32-column groups.  Measured ~1.8-4x PE throughput vs serial.
  - LeakyReLU evac: single ScalarE activation (Lrelu, alpha=0.01, bias) from
    PSUM into q's strided pixel cells; K-stack upcopy via gpsimd-issued DMA.
  - Fusion tail: exp on ACT, eta*patch products on DVE against host-prepared
    pre-shifted mem_stab tap stacks resident in SBUF (no per-strip DMA),
    partition-group sums via 4-wide packed identity matmuls, recip+mul DVE.
"""

import numpy as np
from contextlib import ExitStack

import concourse.bacc as bacc
import concourse.tile as tile
from concourse import mybir
from concourse.bass_utils import run_bass_kernel_spmd

F32 = mybir.dt.float32
BF16 = mybir.dt.bfloat16
ALU = mybir.AluOpType
ACTF = mybir.ActivationFunctionType

H = 128
ST = 129                      # padded row stride
NCOL = ST * 130 + 2           # 16772 sbuf cols
MUCOL = NCOL + 2 * ST + 2
RPS = 3                       # rows per strip

# taps in fusion/unfold order p = 3*kh + kw -> offset 129*(kh-1) + (kw-1)
P_TAPS = [ST * (kh - 1) + (kw - 1) for kh in range(3) for kw in range(3)]


def _j0(r0):
    return ST * (r0 + 1) + 1


def _strips():
    out, r0 = [], 0
    while r0 < H:
        nr = min(RPS, H - r0)
        out.append((r0, nr))
        r0 += nr
    return out


def _pairs():
    s = _strips()
    out, i = [], 0
    while i < len(s):
        if i + 1 < len(s) and s[i + 1][1] == RPS:
            out.append((s[i], s[i + 1]))
            i += 2
        else:
            out.append((s[i],))
            i += 1
    return out


def _build_program(debug=False):
    nc = bacc.Bacc("TRN2", target_bir_lowering=False, debug=False)

    d_xpad = nc.dram_tensor("xpad", [128, NCOL], BF16, kind="ExternalInput")
    d_mu3 = nc.dram_tensor("mu3", [96, MUCOL], BF16, kind="ExternalInput")
    d_msa = nc.dram_tensor("msa", [128, NCOL], BF16, kind="ExternalInput")
    d_msb = nc.dram_tensor("msb", [128, NCOL], BF16, kind="ExternalInput")
    d_ms8 = nc.dram_tensor("ms8", [32, NCOL], BF16, kind="ExternalInput")
    d_w0c1 = nc.dram_tensor("w0c1", [128, 9 * 64], BF16, kind="ExternalInput")
    d_w0c2 = nc.dram_tensor("w0c2", [96, 3 * 64], BF16, kind="ExternalInput")
    d_w1P = nc.dram_tensor("w1P", [128, 3 * 64], BF16, kind="ExternalInput")
    d_w1S = nc.dram_tensor("w1S", [64, 3 * 64], BF16, kind="ExternalInput")
    d_w2P = nc.dram_tensor("w2P", [128, 3 * 64], BF16, kind="ExternalInput")
    d_w2S = nc.dram_tensor("w2S", [64, 3 * 64], BF16, kind="ExternalInput")
    d_wlP = nc.dram_tensor("wlP", [128, 3 * 288], BF16, kind="ExternalInput")
    d_wlS2 = nc.dram_tensor("wlS2", [128, 3 * 288], BF16, kind="ExternalInput")
    d_b = nc.dram_tensor("bias", [128, 3], F32, kind="ExternalInput")
    d_blp = nc.dram_tensor("blp", [128, 4], F32, kind="ExternalInput")
    d_eye = nc.dram_tensor("eye", [128, 32], BF16, kind="ExternalInput")
    d_out = nc.dram_tensor("out", [32, H, H], F32, kind="ExternalOutput")
    if debug:
        d_q1 = nc.dram_tensor("dbg_q1", [128, NCOL], F32, kind="ExternalOutput")
        d_q2 = nc.dram_tensor("dbg_q2", [128, NCOL], F32, kind="ExternalOutput")
        d_q3 = nc.dram_tensor("dbg_q3", [128, NCOL], F32, kind="ExternalOutput")

    pairs = _pairs()
    strips = _strips()

    with tile.TileContext(nc) as tc, ExitStack() as ctx:
        wp = ctx.enter_context(tc.tile_pool(name="wp", bufs=1))
        big = ctx.enter_context(tc.tile_pool(name="big", bufs=1))
        xs = ctx.enter_context(tc.tile_pool(name="xs", bufs=2))
        fu = ctx.enter_context(tc.tile_pool(name="fu", bufs=2))
        f1 = ctx.enter_context(tc.tile_pool(name="f1", bufs=1))
        pm = ctx.enter_context(tc.tile_pool(name="pm", bufs=2, space="PSUM"))
        pA = ctx.enter_context(tc.tile_pool(name="pA", bufs=2, space="PSUM"))
        pB = ctx.enter_context(tc.tile_pool(name="pB", bufs=2, space="PSUM"))
        pD = ctx.enter_context(tc.tile_pool(name="pD", bufs=2, space="PSUM"))

        # ---- weights / constants / resident stacks to SBUF ----
        w0c1 = wp.tile([128, 9 * 64], BF16)
        w0c2 = wp.tile([96, 3 * 64], BF16)
        w1P = wp.tile([128, 3 * 64], BF16)
        w1S = wp.tile([64, 3 * 64], BF16)
        w2P = wp.tile([128, 3 * 64], BF16)
        w2S = wp.tile([64, 3 * 64], BF16)
        wlP = wp.tile([128, 3 * 288], BF16)
        wlS2 = wp.tile([128, 3 * 288], BF16)
        bias = wp.tile([128, 3], F32)
        blp = wp.tile([128, 4], F32)
        eye = wp.tile([128, 32], BF16)
        msa = wp.tile([128, NCOL], BF16)
        msb = wp.tile([128, NCOL], BF16)
        ms8 = wp.tile([32, NCOL], BF16)
        wl_eng = (nc.sync, nc.gpsimd, nc.scalar)
        for k, (dst, src) in enumerate(
                ((w0c1, d_w0c1), (w0c2, d_w0c2), (w1P, d_w1P),
                 (w1S, d_w1S), (w2P, d_w2P), (w2S, d_w2S),
                 (wlP, d_wlP), (wlS2, d_wlS2), (eye, d_eye),
                 (bias, d_b), (blp, d_blp))):
            wl_eng[k % 3].dma_start(out=dst[:], in_=src.ap())

        def wsl(wt, i, m0, mw, step=64):
            return wt[:, i * step + m0: i * step + m0 + mw]

        def new_q(tag):
            q = big.tile([128, NCOL], BF16, tag=tag)
            # zero the pad structure (lower half: head, inter-row cells, tail;
            # upper half: head cell + tail region never covered by upcopies)
            nc.gpsimd.memset(q[0:64, 0:130], 0.0)
            inter = q[0:64, 258:258 + 127 * ST].rearrange(
                "p (m s) -> p m s", s=ST)[:, :, 0:1]
            nc.gpsimd.memset(inter, 0.0)
            nc.gpsimd.memset(q[0:64, ST * 129:NCOL], 0.0)
            last_j0, last_n = _j0(strips[-1][0]), strips[-1][1] * ST
            nc.gpsimd.memset(q[64:128, 0:1], 0.0)
            nc.gpsimd.memset(q[64:128, last_j0 - ST + last_n:NCOL], 0.0)
            return q

        def evac(ps, q, pr, bcol, up_sync=False):
            # ps[64*i : 64*i+64] holds strip i's 64ch: leaky-relu contiguous
            # into q (strip 0 on ACT, strip 1 on DVE to balance engines),
            # re-zero the 3 in-strip pad cells, then K-stack upcopy DMA.
            for i, (r0, nr) in enumerate(pr):
                j0, n = _j0(r0), ST * nr
                if i == 0:
                    nc.scalar.activation(q[0:64, j0:j0 + n], ps[0:64, 0:n],
                                         ACTF.Lrelu,
                                         bias=bias[0:64, bcol:bcol + 1],
                                         alpha=0.01)
                else:
                    tmp = fu.tile([64, 3 * ST], F32, tag="lrtmp")
                    nc.vector.tensor_scalar(tmp[:, 0:n], ps[64:128, 0:n],
                                            bias[64:128, bcol:bcol + 1], None,
                                            ALU.add)
                    nc.vector.scalar_tensor_tensor(q[0:64, j0:j0 + n],
                                                   tmp[:, 0:n], 0.01,
                                                   tmp[:, 0:n],
                                                   op0=ALU.mult, op1=ALU.max)
                pv = q[0:64, j0 + 128:j0 + 128 + nr * ST].rearrange(
                    "p (m s) -> p m s", s=ST)[:, :, 0:1]
                nc.gpsimd.memset(pv, 0.0)
                eng = nc.sync if up_sync else nc.gpsimd
                eng.dma_start(out=q[64:128, j0 - ST:j0 - ST + n],
                              in_=q[0:64, j0:j0 + n])

        # ================= conv0 (streamed input, strip-pair groups) ======
        q1 = new_q("A")
        for pr in pairs:
            r0g = pr[0][0]
            nrg = sum(nr for _, nr in pr)
            jg = _j0(r0g)
            win = ST * nrg + 260
            x1 = xs.tile([128, ST * 6 + 260], BF16, tag="x1")
            x2 = xs.tile([96, ST * 6 + 260], BF16, tag="x2")
            nc.scalar.dma_start(out=x1[:, 0:win],
                                in_=d_xpad.ap()[:, jg - 130:jg - 130 + win])
            nc.sync.dma_start(out=x2[:, 0:win],
                              in_=d_mu3.ap()[:, jg - 130:jg - 130 + win])
            ps = pm.tile([128, 3 * ST], F32, tag="pm")
            off = [(a, b) for a in (-1, 0, 1) for b in (-1, 0, 1)]
            for t, (dr, dc) in enumerate(off):
                for i, (r0, nr) in enumerate(pr):
                    loc = _j0(r0) - jg + 130
                    o = loc + ST * dr + dc
                    nc.tensor.matmul(ps[64 * i:64 * i + 64, 0:ST * nr],
                                     wsl(w0c1, t, 0, 64), x1[:, o:o + ST * nr],
                                     start=(t == 0), stop=False)
            for t, dc in enumerate((-1, 0, 1)):
                for i, (r0, nr) in enumerate(pr):
                    loc = _j0(r0) - jg + 130
                    o = loc - ST + dc
                    nc.tensor.matmul(ps[64 * i:64 * i + 64, 0:ST * nr],
                                     wsl(w0c2, t, 0, 64), x2[:, o:o + ST * nr],
                                     start=False, stop=(t == 2))
            evac(ps, q1, pr, 0)
        # resident mem_stab tap stacks: defer past conv0 (the scheduler
        # hoists dependency-free DMAs to t=0, starving conv0's staging) and
        # chunk with staggered waits so conv1/conv2 upcopy DMAs can slip
        # between chunks instead of queuing behind one 13MB transfer
        CH = (NCOL + 3) // 4
        k = 0
        for dst, src in ((msa, d_msa), (msb, d_msb), (ms8, d_ms8)):
            for c0 in range(0, NCOL, CH):
                c1 = min(c0 + CH, NCOL)
                with tc.tile_wait_until(0.048 + 0.004 * k):
                    nc.gpsimd.dma_start(out=dst[:, c0:c1],
                                        in_=src.ap()[:, c0:c1])
                k += 1
        if debug:
            nc.sync.dma_start(out=d_q1.ap(), in_=q1[:])

        # ================= conv1 / conv2 =================
        def mid_conv(qin, qout, wP, wS, bcol):
            for pr in pairs:
                ps = pm.tile([128, 3 * ST], F32, tag="pm")
                for t, dc in enumerate((-1, 0, 1)):
                    for i, (r0, nr) in enumerate(pr):
                        o = _j0(r0) - ST + dc
                        nc.tensor.matmul(ps[64 * i:64 * i + 64, 0:ST * nr],
                                         wsl(wP, t, 0, 64), qin[0:128, o:o + ST * nr],
                                         start=(t == 0), stop=False)
                for t, dc in enumerate((-1, 0, 1)):
                    for i, (r0, nr) in enumerate(pr):
                        o = _j0(r0) + ST + dc
                        nc.tensor.matmul(ps[64 * i:64 * i + 64, 0:ST * nr],
                                         wsl(wS, t, 0, 64), qin[0:64, o:o + ST * nr],
                                         start=False, stop=(t == 2))
                evac(ps, qout, pr, bcol, up_sync=True)

        q2 = new_q("B")
        mid_conv(q1, q2, w1P, w1S, 1)
        if debug:
            nc.sync.dma_start(out=d_q2.ap(), in_=q2[:])
        q3 = new_q("A")
        mid_conv(q2, q3, w2P, w2S, 2)
        if debug:
            nc.sync.dma_start(out=d_q3.ap(), in_=q3[:])

        # ================= conv_last + softmax + fusion =================
        # software-pipelined one pair deep: pair k's reduction matmuls are
        # emitted after pair k+1's conv_last matmuls, so the PE queue never
        # stalls on the exp->mul chain.
        def tail_compute(pr):
            np_ = len(pr)
            j0s = [_j0(r0) for r0, _ in pr]
            ns = [ST * nr for _, nr in pr]
            # z loads (no deps -> issue early)
            rz = [fu.tile([64, 3 * ST], BF16, tag=f"rz{i}", name=f"rz{i}")
                  for i in range(np_)]
            for i in range(np_):
                nc.sync.dma_start(out=rz[i][32:64, 0:ns[i]],
                                  in_=d_xpad.ap()[64:96, j0s[i]:j0s[i] + ns[i]])
            ppool = (pA, pB)
            ea, eb, ec = [], [], []
            # chunk 0 (head channels 0:128) then chunk 1 (128:256)
            for ck, (m0, edst) in enumerate(((0, ea), (128, eb))):
                ph = [ppool[i].tile([128, 3 * ST], F32, tag=f"p{'AB'[i]}",
                                    name=f"ph{i}")
                      for i in range(np_)]
                for t, dc in enumerate((-1, 0, 1)):
                    for i in range(np_):
                        o = j0s[i] - ST + dc
                        nc.tensor.matmul(ph[i][:, 0:ns[i]],
                                         wsl(wlP, t, m0, 128, 288),
                                         q3[0:128, o:o + ns[i]],
                                         start=(t == 0), stop=False)
                for t, dc in enumerate((-1, 0, 1)):
                    # row-paired K=64 taps: strip0 rows 0:64, strip1 64:128
                    for i in range(np_):
                        if i == 0:
                            lhs = wsl(wlS2, t, m0, 128, 288)[0:64]
                            rhs = q3[0:64, j0s[0] + ST + dc:j0s[0] + ST + dc + ns[0]]
                        else:
                            lhs = wsl(wlS2, t, m0, 128, 288)[64:128]
                            rhs = q3[64:128, j0s[1] + dc:j0s[1] + dc + ns[1]]
                        nc.tensor.matmul(ph[i][:, 0:ns[i]], lhs, rhs,
                                         start=False, stop=(t == 2))
                for i in range(np_):
                    e = fu.tile([128, 3 * ST], BF16, tag=f"e{ck}{i}")
                    nc.scalar.activation(e[:, 0:ns[i]], ph[i][:, 0:ns[i]],
                                         ACTF.Exp, bias=blp[:, ck:ck + 1])
                    edst.append(e)
            # chunk 2 (M=32, both strips col-packed into one [64,.] psum);
            # borrows the pm pool (idle in the tail phase) for bufs=2
            phc = pm.tile([128, 3 * ST], F32, tag="pm")
            for t, dc in enumerate((-1, 0, 1)):
                for i in range(np_):
                    o = j0s[i] - ST + dc
                    nc.tensor.matmul(phc[32 * i:32 * i + 32, 0:ns[i]],
                                     wsl(wlP, t, 256, 32, 288),
                                     q3[0:128, o:o + ns[i]],
                                     start=(t == 0), stop=False)
            for t, dc in enumerate((-1, 0, 1)):
                for i in range(np_):
                    o = j0s[i] + ST + dc
                    nc.tensor.matmul(phc[32 * i:32 * i + 32, 0:ns[i]],
                                     wsl(wlS2, t, 256, 32, 288)[0:64],
                                     q3[0:64, o:o + ns[i]],
                                     start=False, stop=(t == 2))
            for i in range(np_):
                e = fu.tile([32, 3 * ST], BF16, tag=f"ec{i}")
                nc.scalar.activation(e[:, 0:ns[i]],
                                     phc[32 * i:32 * i + 32, 0:ns[i]],
                                     ACTF.Exp, bias=blp[32 * i:32 * i + 32, 3:4])
                ec.append(e)
            # eta * patch products (DVE, same-base operands)
            ta, tb = [], []
            for i in range(np_):
                t1 = fu.tile([128, 3 * ST], BF16, tag=f"ta{i}")
                t2 = fu.tile([128, 3 * ST], BF16, tag=f"tb{i}")
                nc.vector.tensor_mul(t1[:, 0:ns[i]], ea[i][:, 0:ns[i]],
                                     msa[:, j0s[i]:j0s[i] + ns[i]])
                nc.vector.tensor_mul(t2[:, 0:ns[i]], eb[i][:, 0:ns[i]],
                                     msb[:, j0s[i]:j0s[i] + ns[i]])
                nc.vector.tensor_mul(rz[i][0:32, 0:ns[i]], ec[i][:, 0:ns[i]],
                                     ms8[:, j0s[i]:j0s[i] + ns[i]])
                ta.append(t1)
                tb.append(t2)
            return (pr, j0s, ns, rz, ea, eb, ec, ta, tb)

        def tail_fusion(state):
            pr, j0s, ns, rz, ea, eb, ec, ta, tb = state
            np_ = len(pr)
            # packed reduction matmuls: num strip i -> nd[32i:32i+32],
            # den strip i -> nd[64+32i : 96+32i]
            nd = pD.tile([128, 3 * ST], F32, tag="pD")
            for t in range(3):
                for i in range(np_):
                    npos = 32 * i
                    dpos = 64 + 32 * i
                    nl, nr_ = ((eye[:], ta[i]), (eye[:], tb[i]),
                               (eye[0:64], rz[i]))[t]
                    dl, dr = ((eye[:], ea[i]), (eye[:], eb[i]),
                              (eye[0:32], ec[i]))[t]
                    nc.tensor.matmul(nd[npos:npos + 32, 0:ns[i]], nl,
                                     nr_[:, 0:ns[i]], start=(t == 0),
                                     stop=(t == 2), tile_position=(0, npos))
                    nc.tensor.matmul(nd[dpos:dpos + 32, 0:ns[i]], dl,
                                     dr[:, 0:ns[i]], start=(t == 0),
                                     stop=(t == 2), tile_position=(0, dpos))
            # rde = 1/(den+1); ost = num * rde  (both strips at once)
            w = 32 * np_
            den = f1.tile([64, 3 * ST], F32, tag="den")
            rde = f1.tile([64, 3 * ST], F32, tag="rde")
            ost = f1.tile([64, 3 * ST], F32, tag="ost")
            nmax = max(ns)
            nc.vector.tensor_scalar_add(den[0:w, 0:nmax],
                                        nd[64:64 + w, 0:nmax], 1.0)
            nc.vector.reciprocal_approx_fast(rde[0:w, 0:nmax], den[0:w, 0:nmax])
            nc.vector.tensor_mul(ost[0:w, 0:nmax], nd[0:w, 0:nmax],
                                 rde[0:w, 0:nmax])
            for i, (r0, nr) in enumerate(pr):
                src = ost[32 * i:32 * i + 32, 0:ns[i]].rearrange(
                    "p (r c) -> p r c", c=ST)[:, :, 0:128]
                nc.sync.dma_start(out=d_out.ap()[:, r0:r0 + nr, :], in_=src)

        prev = None
        for pr in pairs:
            st = tail_compute(pr)
            if prev is not None:
                tail_fusion(prev)
            prev = st
        tail_fusion(prev)

    nc.compile()
    return nc


BF16_NP = mybir.dt.np(mybir.dt.bfloat16)


def _pad_rows(x, cols):
    # x: [C, 128, 128] -> zero-padded flat rows [C, cols], bf16
    c = x.shape[0]
    buf = np.zeros((c, cols), dtype=BF16_NP)
    buf[:, 130:130 + ST * 128].reshape(c, 128, ST)[:, :, 0:128] = x.astype(BF16_NP)
    return buf


def _shift_stack(flat, offs):
    # flat: [32, NCOL]; returns [32*len(offs), NCOL] rows shifted by offs
    ext = np.zeros((flat.shape[0], NCOL + 264), dtype=flat.dtype)
    ext[:, 132:132 + NCOL] = flat
    return np.concatenate([ext[:, 132 + o:132 + o + NCOL] for o in offs], axis=0)


def _prep_shared(w0, b0, w1, b1, w2, b2, w_last, b_last):
    f = np.float32
    w0t = np.transpose(np.asarray(w0, f), (1, 2, 3, 0))      # [160,3,3,64]
    w0c1 = np.ascontiguousarray(w0t[0:128].reshape(128, 9 * 64))
    w0c2 = np.ascontiguousarray(
        np.transpose(w0t[128:160], (1, 0, 2, 3)).reshape(96, 3 * 64))

    def mid(w):
        wt = np.transpose(np.asarray(w, f), (1, 2, 3, 0))    # [64,3,3,64]
        wP = np.ascontiguousarray(
            np.concatenate([wt[:, 0], wt[:, 1]], 0).reshape(128, 3 * 64))
        wS = np.ascontiguousarray(wt[:, 2].reshape(64, 3 * 64))
        return wP, wS

    w1P, w1S = mid(w1)
    w2P, w2S = mid(w2)
    perm = np.array([(pp % 32) * 9 + pp // 32 for pp in range(288)])
    wl2 = np.asarray(w_last, f)[perm]                        # [288,64,3,3]
    wlt = np.transpose(wl2, (1, 2, 3, 0))                    # [64,3,3,288]
    wlP = np.ascontiguousarray(
        np.concatenate([wlt[:, 0], wlt[:, 1]], 0).reshape(128, 3 * 288))
    wlS = np.ascontiguousarray(wlt[:, 2].reshape(64, 3 * 288))
    wlS2 = np.concatenate([wlS, wlS], axis=0)                # [128, 864]
    bias = np.stack([np.asarray(b0, f), np.asarray(b1, f),
                     np.asarray(b2, f)], axis=1)             # [64, 3]
    bias = np.tile(bias, (2, 1))                             # [128, 3] dup
    blf = np.asarray(b_last, f)[perm]
    blp = np.zeros((128, 4), f)
    blp[:, 0] = blf[0:128]
    blp[:, 1] = blf[128:256]
    blp[0:64, 3] = np.tile(blf[256:288], 2)
    eye = np.tile(np.eye(32, dtype=f), (4, 1))
    out = dict(w0c1=w0c1, w0c2=w0c2, w1P=w1P, w1S=w1S, w2P=w2P, w2S=w2S,
               wlP=wlP, wlS2=wlS2, eye=eye)
    out = {k: v.astype(BF16_NP) for k, v in out.items()}
    out["bias"] = np.ascontiguousarray(bias)
    out["blp"] = blp
    return out


def make_in_maps(z, backbone, mem_stab, mem_unstab, shared):
    f = np.float32
    z = np.asarray(z, f)
    backbone = np.asarray(backbone, f)
    ms = np.asarray(mem_stab, f)
    mu = np.asarray(mem_unstab, f)
    maps = []
    for b in range(z.shape[0]):
        x160 = np.concatenate([backbone[b], z[b], ms[b]], axis=0)
        msf = _pad_rows(ms[b], NCOL)
        muf = _pad_rows(mu[b], MUCOL)
        mu3 = np.concatenate([muf[:, ST * k:ST * k + MUCOL - 2 * ST - 2]
                              for k in range(3)], axis=0)
        mu3 = np.ascontiguousarray(
            np.pad(mu3, ((0, 0), (0, MUCOL - mu3.shape[1]))))
        maps.append(dict(xpad=_pad_rows(x160, NCOL),
                         mu3=mu3,
                         msa=_shift_stack(msf, P_TAPS[0:4]),
                         msb=_shift_stack(msf, P_TAPS[4:8]),
                         ms8=_shift_stack(msf, P_TAPS[8:9]),
                         **shared))
    return maps


_NC_CACHE = {}


def _get_nc(debug=False):
    if debug not in _NC_CACHE:
        _NC_CACHE[debug] = _build_program(debug)
    return _NC_CACHE[debug]


def kernel(z, backbone, mem_stab, mem_unstab, w0, b0, w1, b1, w2, b2,
           w_last, b_last, fusion_kernel_size):
    assert int(fusion_kernel_size) == 3
    shared = _prep_shared(w0, b0, w1, b1, w2, b2, w_last, b_last)
    in_maps = make_in_maps(z, backbone, mem_stab, mem_unstab, shared)
    nc = _get_nc()
    res = run_bass_kernel_spmd(nc, in_maps, core_ids=list(range(len(in_maps))))
    out = np.stack([r["out"] for r in res.results], axis=0)
    return out.astype(np.float32)


# revision 25
# speedup vs baseline: 1.9483x; 1.0107x over previous
"""Trainium2 Bass kernel for nn_ControlledConvEMAStabilizer.

Pipeline (per batch image, one NeuronCore each, batch-parallel over 8 cores):
  q = cat(backbone, z, mem_stab, mem_unstab)          # 160ch
  q = lrelu(conv3x3(q, w0) + b0)                      # -> 64ch
  q = lrelu(conv3x3(q, w1) + b1)                      # -> 64ch
  q = lrelu(conv3x3(q, w2) + b2)                      # -> 64ch
  head = conv3x3(q, w_last) + b_last                  # -> 288ch = 9 taps x 32ch
  eta  = softmax([head; 0]) over the 9+1 slots
  out  = sum_p unfold(mem_stab)[p] * eta[p] + eta[9] * z

Implementation notes:
  - Feature maps live in SBUF as zero-padded flat rows: image pixel (r,c) at
    column 129*(r+1)+1+c (row stride 129, shared single pad column between
    rows, one pad row top/bottom).  Every 3x3 tap is a pure column offset,
    so convs are PSUM-accumulated matmuls over shifted views.
  - K-stacking: intermediates stored twice in one [128, NCOL] tile:
    partitions 0:64 = q, partitions 64:128 = q shifted +129 (one image row).
    A K=128 matmul applies two vertical taps at once.
  - PE sub-array packing via tile_position: strips processed in PAIRS.
    M=64 convs (conv0/1/2) run both strips' matmuls concurrently in the two
    column halves of the PE array (out partitions 0:64 / 64:128).  conv_last
    K=64 tap matmuls row-pair across strips (rows 0:64 / 64:128); the M=32
    head chunk and the softmax-reduction matmuls pack 2- and 4-wide into
    32-column groups.  Measured ~1.8-4x PE throughput vs serial.
  - LeakyReLU evac: single ScalarE activation (Lrelu, alpha=0.01, bias) from
    PSUM into q's strided pixel cells; K-stack upcopy via gpsimd-issued DMA.
  - Fusion tail: exp on ACT, eta*patch products on DVE against host-prepared
    pre-shifted mem_stab tap stacks resident in SBUF (no per-strip DMA),
    partition-group sums via 4-wide packed identity matmuls, recip+mul DVE.
"""

import numpy as np
from contextlib import ExitStack

import concourse.bacc as bacc
import concourse.tile as tile
from concourse import mybir
from concourse.bass_utils import run_bass_kernel_spmd

F32 = mybir.dt.float32
BF16 = mybir.dt.bfloat16
ALU = mybir.AluOpType
ACTF = mybir.ActivationFunctionType

H = 128
ST = 129                      # padded row stride
NCOL = ST * 130 + 2           # 16772 sbuf cols
MUCOL = NCOL + 2 * ST + 2
RPS = 3                       # rows per strip

# taps in fusion/unfold order p = 3*kh + kw -> offset 129*(kh-1) + (kw-1)
P_TAPS = [ST * (kh - 1) + (kw - 1) for kh in range(3) for kw in range(3)]


def _j0(r0):
    return ST * (r0 + 1) + 1


def _strips():
    out, r0 = [], 0
    while r0 < H:
        nr = min(RPS, H - r0)
        out.append((r0, nr))
        r0 += nr
    return out


def _pairs():
    s = _strips()
    out, i = [], 0
    while i < len(s):
        if i + 1 < len(s) and s[i + 1][1] == RPS:
            out.append((s[i], s[i + 1]))
            i += 2
        else:
            out.append((s[i],))
            i += 1
    return out


def _build_program(debug=False):
    nc = bacc.Bacc("TRN2", target_bir_lowering=False, debug=False)

    d_xpad = nc.dram_tensor("xpad", [128, NCOL], BF16, kind="ExternalInput")
    d_mu3 = nc.dram_tensor("mu3", [96, MUCOL], BF16, kind="ExternalInput")
    d_msa = nc.dram_tensor("msa", [128, NCOL], BF16, kind="ExternalInput")
    d_msb = nc.dram_tensor("msb", [128, NCOL], BF16, kind="ExternalInput")
    d_ms8 = nc.dram_tensor("ms8", [32, NCOL], BF16, kind="ExternalInput")
    d_w0c1 = nc.dram_tensor("w0c1", [128, 9 * 64], BF16, kind="ExternalInput")
    d_w0c2 = nc.dram_tensor("w0c2", [96, 3 * 64], BF16, kind="ExternalInput")
    d_w1P = nc.dram_tensor("w1P", [128, 3 * 64], BF16, kind="ExternalInput")
    d_w1S = nc.dram_tensor("w1S", [64, 3 * 64], BF16, kind="ExternalInput")
    d_w2P = nc.dram_tensor("w2P", [128, 3 * 64], BF16, kind="ExternalInput")
    d_w2S = nc.dram_tensor("w2S", [64, 3 * 64], BF16, kind="ExternalInput")
    d_wlP = nc.dram_tensor("wlP", [128, 3 * 288], BF16, kind="ExternalInput")
    d_wlS2 = nc.dram_tensor("wlS2", [128, 3 * 288], BF16, kind="ExternalInput")
    d_b = nc.dram_tensor("bias", [128, 3], F32, kind="ExternalInput")
    d_blp = nc.dram_tensor("blp", [128, 4], F32, kind="ExternalInput")
    d_eye = nc.dram_tensor("eye", [128, 32], BF16, kind="ExternalInput")
    d_out = nc.dram_tensor("out", [32, H, H], F32, kind="ExternalOutput")
    if debug:
        d_q1 = nc.dram_tensor("dbg_q1", [128, NCOL], F32, kind="ExternalOutput")
        d_q2 = nc.dram_tensor("dbg_q2", [128, NCOL], F32, kind="ExternalOutput")
        d_q3 = nc.dram_tensor("dbg_q3", [128, NCOL], F32, kind="ExternalOutput")

    pairs = _pairs()
    strips = _strips()

    with tile.TileContext(nc) as tc, ExitStack() as ctx:
        wp = ctx.enter_context(tc.tile_pool(name="wp", bufs=1))
        big = ctx.enter_context(tc.tile_pool(name="big", bufs=1))
        xs = ctx.enter_context(tc.tile_pool(name="xs", bufs=2))
        fu = ctx.enter_context(tc.tile_pool(name="fu", bufs=2))
        f1 = ctx.enter_context(tc.tile_pool(name="f1", bufs=1))
        pm = ctx.enter_context(tc.tile_pool(name="pm", bufs=2, space="PSUM"))
        pA = ctx.enter_context(tc.tile_pool(name="pA", bufs=2, space="PSUM"))
        pB = ctx.enter_context(tc.tile_pool(name="pB", bufs=2, space="PSUM"))
        pD = ctx.enter_context(tc.tile_pool(name="pD", bufs=2, space="PSUM"))

        # ---- weights / constants / resident stacks to SBUF ----
        w0c1 = wp.tile([128, 9 * 64], BF16)
        w0c2 = wp.tile([96, 3 * 64], BF16)
        w1P = wp.tile([128, 3 * 64], BF16)
        w1S = wp.tile([64, 3 * 64], BF16)
        w2P = wp.tile([128, 3 * 64], BF16)
        w2S = wp.tile([64, 3 * 64], BF16)
        wlP = wp.tile([128, 3 * 288], BF16)
        wlS2 = wp.tile([128, 3 * 288], BF16)
        bias = wp.tile([128, 3], F32)
        blp = wp.tile([128, 4], F32)
        eye = wp.tile([128, 32], BF16)
        msa = wp.tile([128, NCOL], BF16)
        msb = wp.tile([128, NCOL], BF16)
        ms8 = wp.tile([32, NCOL], BF16)
        wl_eng = (nc.sync, nc.gpsimd, nc.scalar)
        for k, (dst, src) in enumerate(
                ((w0c1, d_w0c1), (w0c2, d_w0c2), (w1P, d_w1P),
                 (w1S, d_w1S), (w2P, d_w2P), (w2S, d_w2S),
                 (wlP, d_wlP), (wlS2, d_wlS2), (eye, d_eye),
                 (bias, d_b), (blp, d_blp))):
            wl_eng[k % 3].dma_start(out=dst[:], in_=src.ap())

        def wsl(wt, i, m0, mw, step=64):
            return wt[:, i * step + m0: i * step + m0 + mw]

        def new_q(tag):
            q = big.tile([128, NCOL], BF16, tag=tag)
            # zero the pad structure (lower half: head, inter-row cells, tail;
            # upper half: head cell + tail region never covered by upcopies)
            nc.gpsimd.memset(q[0:64, 0:130], 0.0)
            inter = q[0:64, 258:258 + 127 * ST].rearrange(
                "p (m s) -> p m s", s=ST)[:, :, 0:1]
            nc.gpsimd.memset(inter, 0.0)
            nc.gpsimd.memset(q[0:64, ST * 129:NCOL], 0.0)
            last_j0, last_n = _j0(strips[-1][0]), strips[-1][1] * ST
            nc.gpsimd.memset(q[64:128, 0:1], 0.0)
            nc.gpsimd.memset(q[64:128, last_j0 - ST + last_n:NCOL], 0.0)
            return q

        def evac(ps, q, pr, bcol, mid=False):
            # ps[64*i : 64*i+64] holds strip i's 64ch: leaky-relu contiguous
            # into q, re-zero the 3 in-strip pad cells, K-stack upcopy.
            # Mid convs keep the upcopy OFF the DMA rings (DVE tensor_copy)
            # so the resident-stack transfers can't stall the conv chain.
            for i, (r0, nr) in enumerate(pr):
                j0, n = _j0(r0), ST * nr
                if mid or i == 0:
                    nc.scalar.activation(q[0:64, j0:j0 + n],
                                         ps[64 * i:64 * i + 64, 0:n],
                                         ACTF.Lrelu,
                                         bias=bias[64 * i:64 * i + 64,
                                                   bcol:bcol + 1],
                                         alpha=0.01)
                else:
                    tmp = fu.tile([64, 3 * ST], F32, tag="lrtmp")
                    nc.vector.tensor_scalar(tmp[:, 0:n], ps[64:128, 0:n],
                                            bias[64:128, bcol:bcol + 1], None,
                                            ALU.add)
                    nc.vector.scalar_tensor_tensor(q[0:64, j0:j0 + n],
                                                   tmp[:, 0:n], 0.01,
                                                   tmp[:, 0:n],
                                                   op0=ALU.mult, op1=ALU.max)
                pv = q[0:64, j0 + 128:j0 + 128 + nr * ST].rearrange(
                    "p (m s) -> p m s", s=ST)[:, :, 0:1]
                nc.gpsimd.memset(pv, 0.0)
                if mid:
                    nc.vector.tensor_copy(q[64:128, j0 - ST:j0 - ST + n],
                                          q[0:64, j0:j0 + n])
                else:
                    nc.gpsimd.dma_start(out=q[64:128, j0 - ST:j0 - ST + n],
                                        in_=q[0:64, j0:j0 + n])

        # ================= conv0 (streamed input, strip-pair groups) ======
        q1 = new_q("A")
        for pr in pairs:
            r0g = pr[0][0]
            nrg = sum(nr for _, nr in pr)
            jg = _j0(r0g)
            win = ST * nrg + 260
            x1 = xs.tile([128, ST * 6 + 260], BF16, tag="x1")
            x2 = xs.tile([96, ST * 6 + 260], BF16, tag="x2")
            nc.scalar.dma_start(out=x1[:, 0:win],
                                in_=d_xpad.ap()[:, jg - 130:jg - 130 + win])
            nc.sync.dma_start(out=x2[:, 0:win],
                              in_=d_mu3.ap()[:, jg - 130:jg - 130 + win])
            ps = pm.tile([128, 3 * ST], F32, tag="pm")
            off = [(a, b) for a in (-1, 0, 1) for b in (-1, 0, 1)]
            for t, (dr, dc) in enumerate(off):
                for i, (r0, nr) in enumerate(pr):
                    loc = _j0(r0) - jg + 130
                    o = loc + ST * dr + dc
                    nc.tensor.matmul(ps[64 * i:64 * i + 64, 0:ST * nr],
                                     wsl(w0c1, t, 0, 64), x1[:, o:o + ST * nr],
                                     start=(t == 0), stop=False)
            for t, dc in enumerate((-1, 0, 1)):
                for i, (r0, nr) in enumerate(pr):
                    loc = _j0(r0) - jg + 130
                    o = loc - ST + dc
                    nc.tensor.matmul(ps[64 * i:64 * i + 64, 0:ST * nr],
                                     wsl(w0c2, t, 0, 64), x2[:, o:o + ST * nr],
                                     start=False, stop=(t == 2))
            evac(ps, q1, pr, 0)
        # resident mem_stab tap stacks: defer past conv0 (the scheduler
        # hoists dependency-free DMAs to t=0, starving conv0's staging) and
        # chunk with staggered waits so conv1/conv2 upcopy DMAs can slip
        # between chunks instead of queuing behind one 13MB transfer
        CH = (NCOL + 3) // 4
        k = 0
        for dst, src in ((msa, d_msa), (msb, d_msb), (ms8, d_ms8)):
            for c0 in range(0, NCOL, CH):
                c1 = min(c0 + CH, NCOL)
                with tc.tile_wait_until(0.048 + 0.004 * k):
                    nc.gpsimd.dma_start(out=dst[:, c0:c1],
                                        in_=src.ap()[:, c0:c1])
                k += 1
        if debug:
            nc.sync.dma_start(out=d_q1.ap(), in_=q1[:])

        # ================= conv1 / conv2 =================
        def mid_conv(qin, qout, wP, wS, bcol):
            for pr in pairs:
                ps = pm.tile([128, 3 * ST], F32, tag="pm")
                for t, dc in enumerate((-1, 0, 1)):
                    for i, (r0, nr) in enumerate(pr):
                        o = _j0(r0) - ST + dc
                        nc.tensor.matmul(ps[64 * i:64 * i + 64, 0:ST * nr],
                                         wsl(wP, t, 0, 64), qin[0:128, o:o + ST * nr],
                                         start=(t == 0), stop=False)
                for t, dc in enumerate((-1, 0, 1)):
                    for i, (r0, nr) in enumerate(pr):
                        o = _j0(r0) + ST + dc
                        nc.tensor.matmul(ps[64 * i:64 * i + 64, 0:ST * nr],
                                         wsl(wS, t, 0, 64), qin[0:64, o:o + ST * nr],
                                         start=False, stop=(t == 2))
                evac(ps, qout, pr, bcol, mid=True)

        q2 = new_q("B")
        mid_conv(q1, q2, w1P, w1S, 1)
        if debug:
            nc.sync.dma_start(out=d_q2.ap(), in_=q2[:])
        q3 = new_q("A")
        mid_conv(q2, q3, w2P, w2S, 2)
        if debug:
            nc.sync.dma_start(out=d_q3.ap(), in_=q3[:])

        # ================= conv_last + softmax + fusion =================
        # software-pipelined one pair deep: pair k's reduction matmuls are
        # emitted after pair k+1's conv_last matmuls, so the PE queue never
        # stalls on the exp->mul chain.
        def tail_compute(pr):
            np_ = len(pr)
            j0s = [_j0(r0) for r0, _ in pr]
            ns = [ST * nr for _, nr in pr]
            # z loads (no deps -> issue early)
            rz = [fu.tile([64, 3 * ST], BF16, tag=f"rz{i}", name=f"rz{i}")
                  for i in range(np_)]
            for i in range(np_):
                nc.sync.dma_start(out=rz[i][32:64, 0:ns[i]],
                                  in_=d_xpad.ap()[64:96, j0s[i]:j0s[i] + ns[i]])
            ppool = (pA, pB)
            ea, eb, ec = [], [], []
            # chunk 0 (head channels 0:128) then chunk 1 (128:256)
            for ck, (m0, edst) in enumerate(((0, ea), (128, eb))):
                ph = [ppool[i].tile([128, 3 * ST], F32, tag=f"p{'AB'[i]}",
                                    name=f"ph{i}")
                      for i in range(np_)]
                for t, dc in enumerate((-1, 0, 1)):
                    for i in range(np_):
                        o = j0s[i] - ST + dc
                        nc.tensor.matmul(ph[i][:, 0:ns[i]],
                                         wsl(wlP, t, m0, 128, 288),
                                         q3[0:128, o:o + ns[i]],
                                         start=(t == 0), stop=False)
                for t, dc in enumerate((-1, 0, 1)):
                    # row-paired K=64 taps: strip0 rows 0:64, strip1 64:128
                    for i in range(np_):
                        if i == 0:
                            lhs = wsl(wlS2, t, m0, 128, 288)[0:64]
                            rhs = q3[0:64, j0s[0] + ST + dc:j0s[0] + ST + dc + ns[0]]
                        else:
                            lhs = wsl(wlS2, t, m0, 128, 288)[64:128]
                            rhs = q3[64:128, j0s[1] + dc:j0s[1] + dc + ns[1]]
                        nc.tensor.matmul(ph[i][:, 0:ns[i]], lhs, rhs,
                                         start=False, stop=(t == 2))
                for i in range(np_):
                    e = fu.tile([128, 3 * ST], BF16, tag=f"e{ck}{i}")
                    nc.scalar.activation(e[:, 0:ns[i]], ph[i][:, 0:ns[i]],
                                         ACTF.Exp, bias=blp[:, ck:ck + 1])
                    edst.append(e)
            # chunk 2 (M=32, both strips col-packed into one [64,.] psum);
            # borrows the pm pool (idle in the tail phase) for bufs=2
            phc = pm.tile([128, 3 * ST], F32, tag="pm")
            for t, dc in enumerate((-1, 0, 1)):
                for i in range(np_):
                    o = j0s[i] - ST + dc
                    nc.tensor.matmul(phc[32 * i:32 * i + 32, 0:ns[i]],
                                     wsl(wlP, t, 256, 32, 288),
                                     q3[0:128, o:o + ns[i]],
                                     start=(t == 0), stop=False)
            for t, dc in enumerate((-1, 0, 1)):
                for i in range(np_):
                    o = j0s[i] + ST + dc
                    nc.tensor.matmul(phc[32 * i:32 * i + 32, 0:ns[i]],
                                     wsl(wlS2, t, 256, 32, 288)[0:64],
                                     q3[0:64, o:o + ns[i]],
                                     start=False, stop=(t == 2))
            for i in range(np_):
                e = fu.tile([32, 3 * ST], BF16, tag=f"ec{i}")
                nc.scalar.activation(e[:, 0:ns[i]],
                                     phc[32 * i:32 * i + 32, 0:ns[i]],
                                     ACTF.Exp, bias=blp[32 * i:32 * i + 32, 3:4])
                ec.append(e)
            # eta * patch products (DVE, same-base operands)
            ta, tb = [], []
            for i in range(np_):
                t1 = fu.tile([128, 3 * ST], BF16, tag=f"ta{i}")
                t2 = fu.tile([128, 3 * ST], BF16, tag=f"tb{i}")
                nc.vector.tensor_mul(t1[:, 0:ns[i]], ea[i][:, 0:ns[i]],
                                     msa[:, j0s[i]:j0s[i] + ns[i]])
                nc.vector.tensor_mul(t2[:, 0:ns[i]], eb[i][:, 0:ns[i]],
                                     msb[:, j0s[i]:j0s[i] + ns[i]])
                nc.vector.tensor_mul(rz[i][0:32, 0:ns[i]], ec[i][:, 0:ns[i]],
                                     ms8[:, j0s[i]:j0s[i] + ns[i]])
                ta.append(t1)
                tb.append(t2)
            return (pr, j0s, ns, rz, ea, eb, ec, ta, tb)

        def tail_fusion(state):
            pr, j0s, ns, rz, ea, eb, ec, ta, tb = state
            np_ = len(pr)
            # packed reduction matmuls: num strip i -> nd[32i:32i+32],
            # den strip i -> nd[64+32i : 96+32i]
            nd = pD.tile([128, 3 * ST], F32, tag="pD")
            for t in range(3):
                for i in range(np_):
                    npos = 32 * i
                    dpos = 64 + 32 * i
                    nl, nr_ = ((eye[:], ta[i]), (eye[:], tb[i]),
                               (eye[0:64], rz[i]))[t]
                    dl, dr = ((eye[:], ea[i]), (eye[:], eb[i]),
                              (eye[0:32], ec[i]))[t]
                    nc.tensor.matmul(nd[npos:npos + 32, 0:ns[i]], nl,
                                     nr_[:, 0:ns[i]], start=(t == 0),
                                     stop=(t == 2), tile_position=(0, npos))
                    nc.tensor.matmul(nd[dpos:dpos + 32, 0:ns[i]], dl,
                                     dr[:, 0:ns[i]], start=(t == 0),
                                     stop=(t == 2), tile_position=(0, dpos))
            # rde = 1/(den+1); ost = num * rde  (both strips at once)
            w = 32 * np_
            den = f1.tile([64, 3 * ST], F32, tag="den")
            rde = f1.tile([64, 3 * ST], F32, tag="rde")
            ost = f1.tile([64, 3 * ST], F32, tag="ost")
            nmax = max(ns)
            nc.vector.tensor_scalar_add(den[0:w, 0:nmax],
                                        nd[64:64 + w, 0:nmax], 1.0)
            nc.vector.reciprocal_approx_fast(rde[0:w, 0:nmax], den[0:w, 0:nmax])
            nc.vector.tensor_mul(ost[0:w, 0:nmax], nd[0:w, 0:nmax],
                                 rde[0:w, 0:nmax])
            for i, (r0, nr) in enumerate(pr):
                src = ost[32 * i:32 * i + 32, 0:ns[i]].rearrange(
                    "p (r c) -> p r c", c=ST)[:, :, 0:128]
                nc.sync.dma_start(out=d_out.ap()[:, r0:r0 + nr, :], in_=src)

        prev = None
        for pr in pairs:
            st = tail_compute(pr)
            if prev is not None:
                tail_fusion(prev)
            prev = st
        tail_fusion(prev)

    nc.compile()
    return nc


BF16_NP = mybir.dt.np(mybir.dt.bfloat16)


def _pad_rows(x, cols):
    # x: [C, 128, 128] -> zero-padded flat rows [C, cols], bf16
    c = x.shape[0]
    buf = np.zeros((c, cols), dtype=BF16_NP)
    buf[:, 130:130 + ST * 128].reshape(c, 128, ST)[:, :, 0:128] = x.astype(BF16_NP)
    return buf


def _shift_stack(flat, offs):
    # flat: [32, NCOL]; returns [32*len(offs), NCOL] rows shifted by offs
    ext = np.zeros((flat.shape[0], NCOL + 264), dtype=flat.dtype)
    ext[:, 132:132 + NCOL] = flat
    return np.concatenate([ext[:, 132 + o:132 + o + NCOL] for o in offs], axis=0)


def _prep_shared(w0, b0, w1, b1, w2, b2, w_last, b_last):
    f = np.float32
    w0t = np.transpose(np.asarray(w0, f), (1, 2, 3, 0))      # [160,3,3,64]
    w0c1 = np.ascontiguousarray(w0t[0:128].reshape(128, 9 * 64))
    w0c2 = np.ascontiguousarray(
        np.transpose(w0t[128:160], (1, 0, 2, 3)).reshape(96, 3 * 64))

    def mid(w):
        wt = np.transpose(np.asarray(w, f), (1, 2, 3, 0))    # [64,3,3,64]
        wP = np.ascontiguousarray(
            np.concatenate([wt[:, 0], wt[:, 1]], 0).reshape(128, 3 * 64))
        wS = np.ascontiguousarray(wt[:, 2].reshape(64, 3 * 64))
        return wP, wS

    w1P, w1S = mid(w1)
    w2P, w2S = mid(w2)
    perm = np.array([(pp % 32) * 9 + pp // 32 for pp in range(288)])
    wl2 = np.asarray(w_last, f)[perm]                        # [288,64,3,3]
    wlt = np.transpose(wl2, (1, 2, 3, 0))                    # [64,3,3,288]
    wlP = np.ascontiguousarray(
        np.concatenate([wlt[:, 0], wlt[:, 1]], 0).reshape(128, 3 * 288))
    wlS = np.ascontiguousarray(wlt[:, 2].reshape(64, 3 * 288))
    wlS2 = np.concatenate([wlS, wlS], axis=0)                # [128, 864]
    bias = np.stack([np.asarray(b0, f), np.asarray(b1, f),
                     np.asarray(b2, f)], axis=1)             # [64, 3]
    bias = np.tile(bias, (2, 1))                             # [128, 3] dup
    blf = np.asarray(b_last, f)[perm]
    blp = np.zeros((128, 4), f)
    blp[:, 0] = blf[0:128]
    blp[:, 1] = blf[128:256]
    blp[0:64, 3] = np.tile(blf[256:288], 2)
    eye = np.tile(np.eye(32, dtype=f), (4, 1))
    out = dict(w0c1=w0c1, w0c2=w0c2, w1P=w1P, w1S=w1S, w2P=w2P, w2S=w2S,
               wlP=wlP, wlS2=wlS2, eye=eye)
    out = {k: v.astype(BF16_NP) for k, v in out.items()}
    out["bias"] = np.ascontiguousarray(bias)
    out["blp"] = blp
    return out


def make_in_maps(z, backbone, mem_stab, mem_unstab, shared):
    f = np.float32
    z = np.asarray(z, f)
    backbone = np.asarray(backbone, f)
    ms = np.asarray(mem_stab, f)
    mu = np.asarray(mem_unstab, f)
    maps = []
    for b in range(z.shape[0]):
        x160 = np.concatenate([backbone[b], z[b], ms[b]], axis=0)
        msf = _pad_rows(ms[b], NCOL)
        muf = _pad_rows(mu[b], MUCOL)
        mu3 = np.concatenate([muf[:, ST * k:ST * k + MUCOL - 2 * ST - 2]
                              for k in range(3)], axis=0)
        mu3 = np.ascontiguousarray(
            np.pad(mu3, ((0, 0), (0, MUCOL - mu3.shape[1]))))
        maps.append(dict(xpad=_pad_rows(x160, NCOL),
                         mu3=mu3,
                         msa=_shift_stack(msf, P_TAPS[0:4]),
                         msb=_shift_stack(msf, P_TAPS[4:8]),
                         ms8=_shift_stack(msf, P_TAPS[8:9]),
                         **shared))
    return maps


_NC_CACHE = {}


def _get_nc(debug=False):
    if debug not in _NC_CACHE:
        _NC_CACHE[debug] = _build_program(debug)
    return _NC_CACHE[debug]


def kernel(z, backbone, mem_stab, mem_unstab, w0, b0, w1, b1, w2, b2,
           w_last, b_last, fusion_kernel_size):
    assert int(fusion_kernel_size) == 3
    shared = _prep_shared(w0, b0, w1, b1, w2, b2, w_last, b_last)
    in_maps = make_in_maps(z, backbone, mem_stab, mem_unstab, shared)
    nc = _get_nc()
    res = run_bass_kernel_spmd(nc, in_maps, core_ids=list(range(len(in_maps))))
    out = np.stack([r["out"] for r in res.results], axis=0)
    return out.astype(np.float32)


# revision 26
# speedup vs baseline: 1.9794x; 1.0160x over previous
"""Trainium2 Bass kernel for nn_ControlledConvEMAStabilizer.

Pipeline (per batch image, one NeuronCore each, batch-parallel over 8 cores):
  q = cat(backbone, z, mem_stab, mem_unstab)          # 160ch
  q = lrelu(conv3x3(q, w0) + b0)                      # -> 64ch
  q = lrelu(conv3x3(q, w1) + b1)                      # -> 64ch
  q = lrelu(conv3x3(q, w2) + b2)                      # -> 64ch
  head = conv3x3(q, w_last) + b_last                  # -> 288ch = 9 taps x 32ch
  eta  = softmax([head; 0]) over the 9+1 slots
  out  = sum_p unfold(mem_stab)[p] * eta[p] + eta[9] * z

Implementation notes:
  - Feature maps live in SBUF as zero-padded flat rows: image pixel (r,c) at
    column 129*(r+1)+1+c (row stride 129, shared single pad column between
    rows, one pad row top/bottom).  Every 3x3 tap is a pure column offset,
    so convs are PSUM-accumulated matmuls over shifted views.
  - K-stacking: intermediates stored twice in one [128, NCOL] tile:
    partitions 0:64 = q, partitions 64:128 = q shifted +129 (one image row).
    A K=128 matmul applies two vertical taps at once.
  - PE sub-array packing via tile_position: strips processed in PAIRS.
    M=64 convs (conv0/1/2) run both strips' matmuls concurrently in the two
    column halves of the PE array (out partitions 0:64 / 64:128).  conv_last
    K=64 tap matmuls row-pair across strips (rows 0:64 / 64:128); the M=32
    head chunk and the softmax-reduction matmuls pack 2- and 4-wide into
    32-column groups.  Measured ~1.8-4x PE throughput vs serial.
  - LeakyReLU evac: single ScalarE activation (Lrelu, alpha=0.01, bias) from
    PSUM into q's strided pixel cells; K-stack upcopy via gpsimd-issued DMA.
  - Fusion tail: exp on ACT, eta*patch products on DVE against host-prepared
    pre-shifted mem_stab tap stacks resident in SBUF (no per-strip DMA),
    partition-group sums via 4-wide packed identity matmuls, recip+mul DVE.
"""

import numpy as np
from contextlib import ExitStack

import concourse.bacc as bacc
import concourse.tile as tile
from concourse import mybir
from concourse.bass_utils import run_bass_kernel_spmd

F32 = mybir.dt.float32
BF16 = mybir.dt.bfloat16
ALU = mybir.AluOpType
ACTF = mybir.ActivationFunctionType

H = 128
ST = 129                      # padded row stride
NCOL = ST * 130 + 2           # 16772 sbuf cols
MUCOL = NCOL + 2 * ST + 2
RPS = 3                       # rows per strip

# taps in fusion/unfold order p = 3*kh + kw -> offset 129*(kh-1) + (kw-1)
P_TAPS = [ST * (kh - 1) + (kw - 1) for kh in range(3) for kw in range(3)]


def _j0(r0):
    return ST * (r0 + 1) + 1


def _strips():
    out, r0 = [], 0
    while r0 < H:
        nr = min(RPS, H - r0)
        out.append((r0, nr))
        r0 += nr
    return out


def _pairs():
    s = _strips()
    out, i = [], 0
    while i < len(s):
        if i + 1 < len(s) and s[i + 1][1] == RPS:
            out.append((s[i], s[i + 1]))
            i += 2
        else:
            out.append((s[i],))
            i += 1
    return out


def _build_program(debug=False):
    nc = bacc.Bacc("TRN2", target_bir_lowering=False, debug=False)

    d_xpad = nc.dram_tensor("xpad", [128, NCOL], BF16, kind="ExternalInput")
    d_mu3 = nc.dram_tensor("mu3", [96, MUCOL], BF16, kind="ExternalInput")
    d_msa = nc.dram_tensor("msa", [128, NCOL], BF16, kind="ExternalInput")
    d_msb = nc.dram_tensor("msb", [128, NCOL], BF16, kind="ExternalInput")
    d_ms8 = nc.dram_tensor("ms8", [32, NCOL], BF16, kind="ExternalInput")
    d_w0c1 = nc.dram_tensor("w0c1", [128, 9 * 64], BF16, kind="ExternalInput")
    d_w0c2 = nc.dram_tensor("w0c2", [96, 3 * 64], BF16, kind="ExternalInput")
    d_w1P = nc.dram_tensor("w1P", [128, 3 * 64], BF16, kind="ExternalInput")
    d_w1S = nc.dram_tensor("w1S", [64, 3 * 64], BF16, kind="ExternalInput")
    d_w2P = nc.dram_tensor("w2P", [128, 3 * 64], BF16, kind="ExternalInput")
    d_w2S = nc.dram_tensor("w2S", [64, 3 * 64], BF16, kind="ExternalInput")
    d_wlP = nc.dram_tensor("wlP", [128, 3 * 288], BF16, kind="ExternalInput")
    d_wlS2 = nc.dram_tensor("wlS2", [128, 3 * 288], BF16, kind="ExternalInput")
    d_b = nc.dram_tensor("bias", [128, 3], F32, kind="ExternalInput")
    d_blp = nc.dram_tensor("blp", [128, 4], F32, kind="ExternalInput")
    d_eye = nc.dram_tensor("eye", [128, 32], BF16, kind="ExternalInput")
    d_out = nc.dram_tensor("out", [32, H, H], F32, kind="ExternalOutput")
    if debug:
        d_q1 = nc.dram_tensor("dbg_q1", [128, NCOL], F32, kind="ExternalOutput")
        d_q2 = nc.dram_tensor("dbg_q2", [128, NCOL], F32, kind="ExternalOutput")
        d_q3 = nc.dram_tensor("dbg_q3", [128, NCOL], F32, kind="ExternalOutput")

    pairs = _pairs()
    strips = _strips()

    with tile.TileContext(nc) as tc, ExitStack() as ctx:
        wp = ctx.enter_context(tc.tile_pool(name="wp", bufs=1))
        big = ctx.enter_context(tc.tile_pool(name="big", bufs=1))
        xs = ctx.enter_context(tc.tile_pool(name="xs", bufs=2))
        fu = ctx.enter_context(tc.tile_pool(name="fu", bufs=2))
        f1 = ctx.enter_context(tc.tile_pool(name="f1", bufs=1))
        pm = ctx.enter_context(tc.tile_pool(name="pm", bufs=2, space="PSUM"))
        pA = ctx.enter_context(tc.tile_pool(name="pA", bufs=2, space="PSUM"))
        pB = ctx.enter_context(tc.tile_pool(name="pB", bufs=2, space="PSUM"))
        pD = ctx.enter_context(tc.tile_pool(name="pD", bufs=2, space="PSUM"))

        # ---- weights / constants / resident stacks to SBUF ----
        w0c1 = wp.tile([128, 9 * 64], BF16)
        w0c2 = wp.tile([96, 3 * 64], BF16)
        w1P = wp.tile([128, 3 * 64], BF16)
        w1S = wp.tile([64, 3 * 64], BF16)
        w2P = wp.tile([128, 3 * 64], BF16)
        w2S = wp.tile([64, 3 * 64], BF16)
        wlP = wp.tile([128, 3 * 288], BF16)
        wlS2 = wp.tile([128, 3 * 288], BF16)
        bias = wp.tile([128, 3], F32)
        blp = wp.tile([128, 4], F32)
        eye = wp.tile([128, 32], BF16)
        msa = wp.tile([128, NCOL], BF16)
        msb = wp.tile([128, NCOL], BF16)
        ms8 = wp.tile([32, NCOL], BF16)
        wl_eng = (nc.sync, nc.gpsimd, nc.scalar)
        for k, (dst, src) in enumerate(
                ((w0c1, d_w0c1), (w0c2, d_w0c2), (w1P, d_w1P),
                 (w1S, d_w1S), (w2P, d_w2P), (w2S, d_w2S),
                 (wlP, d_wlP), (wlS2, d_wlS2), (eye, d_eye),
                 (bias, d_b), (blp, d_blp))):
            wl_eng[k % 3].dma_start(out=dst[:], in_=src.ap())

        def wsl(wt, i, m0, mw, step=64):
            return wt[:, i * step + m0: i * step + m0 + mw]

        def new_q(tag):
            q = big.tile([128, NCOL], BF16, tag=tag)
            # zero the pad structure (lower half: head, inter-row cells, tail;
            # upper half: head cell + tail region never covered by upcopies)
            nc.gpsimd.memset(q[0:64, 0:130], 0.0)
            inter = q[0:64, 258:258 + 127 * ST].rearrange(
                "p (m s) -> p m s", s=ST)[:, :, 0:1]
            nc.gpsimd.memset(inter, 0.0)
            nc.gpsimd.memset(q[0:64, ST * 129:NCOL], 0.0)
            last_j0, last_n = _j0(strips[-1][0]), strips[-1][1] * ST
            nc.gpsimd.memset(q[64:128, 0:1], 0.0)
            nc.gpsimd.memset(q[64:128, last_j0 - ST + last_n:NCOL], 0.0)
            return q

        def evac(ps, q, pr, bcol, mid=False):
            # ps[64*i : 64*i+64] holds strip i's 64ch: leaky-relu contiguous
            # into q, re-zero the 3 in-strip pad cells, K-stack upcopy.
            # Mid convs keep the upcopy OFF the DMA rings (DVE tensor_copy)
            # so the resident-stack transfers can't stall the conv chain.
            for i, (r0, nr) in enumerate(pr):
                j0, n = _j0(r0), ST * nr
                if mid or i == 0:
                    nc.scalar.activation(q[0:64, j0:j0 + n],
                                         ps[64 * i:64 * i + 64, 0:n],
                                         ACTF.Lrelu,
                                         bias=bias[64 * i:64 * i + 64,
                                                   bcol:bcol + 1],
                                         alpha=0.01)
                else:
                    tmp = fu.tile([64, 3 * ST], F32, tag="lrtmp")
                    nc.vector.tensor_scalar(tmp[:, 0:n], ps[64:128, 0:n],
                                            bias[64:128, bcol:bcol + 1], None,
                                            ALU.add)
                    nc.vector.scalar_tensor_tensor(q[0:64, j0:j0 + n],
                                                   tmp[:, 0:n], 0.01,
                                                   tmp[:, 0:n],
                                                   op0=ALU.mult, op1=ALU.max)
                pv = q[0:64, j0 + 128:j0 + 128 + nr * ST].rearrange(
                    "p (m s) -> p m s", s=ST)[:, :, 0:1]
                nc.gpsimd.memset(pv, 0.0)
                if mid:
                    nc.vector.tensor_copy(q[64:128, j0 - ST:j0 - ST + n],
                                          q[0:64, j0:j0 + n])
                else:
                    nc.gpsimd.dma_start(out=q[64:128, j0 - ST:j0 - ST + n],
                                        in_=q[0:64, j0:j0 + n])

        # ================= conv0 (streamed input, strip-pair groups) ======
        q1 = new_q("A")
        for pr in pairs:
            r0g = pr[0][0]
            nrg = sum(nr for _, nr in pr)
            jg = _j0(r0g)
            win = ST * nrg + 260
            x1 = xs.tile([128, ST * 6 + 260], BF16, tag="x1")
            x2 = xs.tile([96, ST * 6 + 260], BF16, tag="x2")
            nc.scalar.dma_start(out=x1[:, 0:win],
                                in_=d_xpad.ap()[:, jg - 130:jg - 130 + win])
            nc.sync.dma_start(out=x2[:, 0:win],
                              in_=d_mu3.ap()[:, jg - 130:jg - 130 + win])
            ps = pm.tile([128, 3 * ST], F32, tag="pm")
            off = [(a, b) for a in (-1, 0, 1) for b in (-1, 0, 1)]
            for t, (dr, dc) in enumerate(off):
                for i, (r0, nr) in enumerate(pr):
                    loc = _j0(r0) - jg + 130
                    o = loc + ST * dr + dc
                    nc.tensor.matmul(ps[64 * i:64 * i + 64, 0:ST * nr],
                                     wsl(w0c1, t, 0, 64), x1[:, o:o + ST * nr],
                                     start=(t == 0), stop=False)
            for t, dc in enumerate((-1, 0, 1)):
                for i, (r0, nr) in enumerate(pr):
                    loc = _j0(r0) - jg + 130
                    o = loc - ST + dc
                    nc.tensor.matmul(ps[64 * i:64 * i + 64, 0:ST * nr],
                                     wsl(w0c2, t, 0, 64), x2[:, o:o + ST * nr],
                                     start=False, stop=(t == 2))
            evac(ps, q1, pr, 0)
        # resident mem_stab tap stacks: defer past conv0 (the scheduler
        # hoists dependency-free DMAs to t=0, starving conv0's staging) and
        # chunk with staggered waits so conv1/conv2 upcopy DMAs can slip
        # between chunks instead of queuing behind one 13MB transfer
        CH = (NCOL + 3) // 4
        k = 0
        for dst, src in ((msa, d_msa), (msb, d_msb), (ms8, d_ms8)):
            for c0 in range(0, NCOL, CH):
                c1 = min(c0 + CH, NCOL)
                with tc.tile_wait_until(0.048 + 0.004 * k):
                    nc.sync.dma_start(out=dst[:, c0:c1],
                                      in_=src.ap()[:, c0:c1])
                k += 1
        if debug:
            nc.sync.dma_start(out=d_q1.ap(), in_=q1[:])

        # ================= conv1 / conv2 =================
        def mid_conv(qin, qout, wP, wS, bcol):
            for pr in pairs:
                ps = pm.tile([128, 3 * ST], F32, tag="pm")
                for t, dc in enumerate((-1, 0, 1)):
                    for i, (r0, nr) in enumerate(pr):
                        o = _j0(r0) - ST + dc
                        nc.tensor.matmul(ps[64 * i:64 * i + 64, 0:ST * nr],
                                         wsl(wP, t, 0, 64), qin[0:128, o:o + ST * nr],
                                         start=(t == 0), stop=False)
                for t, dc in enumerate((-1, 0, 1)):
                    for i, (r0, nr) in enumerate(pr):
                        o = _j0(r0) + ST + dc
                        nc.tensor.matmul(ps[64 * i:64 * i + 64, 0:ST * nr],
                                         wsl(wS, t, 0, 64), qin[0:64, o:o + ST * nr],
                                         start=False, stop=(t == 2))
                evac(ps, qout, pr, bcol, mid=True)

        q2 = new_q("B")
        mid_conv(q1, q2, w1P, w1S, 1)
        if debug:
            nc.sync.dma_start(out=d_q2.ap(), in_=q2[:])
        q3 = new_q("A")
        mid_conv(q2, q3, w2P, w2S, 2)
        if debug:
            nc.sync.dma_start(out=d_q3.ap(), in_=q3[:])

        # ================= conv_last + softmax + fusion =================
        # software-pipelined one pair deep: pair k's reduction matmuls are
        # emitted after pair k+1's conv_last matmuls, so the PE queue never
        # stalls on the exp->mul chain.
        def tail_compute(pr):
            np_ = len(pr)
            j0s = [_j0(r0) for r0, _ in pr]
            ns = [ST * nr for _, nr in pr]
            # z loads (no deps -> issue early)
            rz = [fu.tile([64, 3 * ST], BF16, tag=f"rz{i}", name=f"rz{i}")
                  for i in range(np_)]
            for i in range(np_):
                nc.sync.dma_start(out=rz[i][32:64, 0:ns[i]],
                                  in_=d_xpad.ap()[64:96, j0s[i]:j0s[i] + ns[i]])
            ppool = (pA, pB)
            ea, eb, ec = [], [], []
            # chunk 0 (head channels 0:128) then chunk 1 (128:256)
            for ck, (m0, edst) in enumerate(((0, ea), (128, eb))):
                ph = [ppool[i].tile([128, 3 * ST], F32, tag=f"p{'AB'[i]}",
                                    name=f"ph{i}")
                      for i in range(np_)]
                for t, dc in enumerate((-1, 0, 1)):
                    for i in range(np_):
                        o = j0s[i] - ST + dc
                        nc.tensor.matmul(ph[i][:, 0:ns[i]],
                                         wsl(wlP, t, m0, 128, 288),
                                         q3[0:128, o:o + ns[i]],
                                         start=(t == 0), stop=False)
                for t, dc in enumerate((-1, 0, 1)):
                    # row-paired K=64 taps: strip0 rows 0:64, strip1 64:128
                    for i in range(np_):
                        if i == 0:
                            lhs = wsl(wlS2, t, m0, 128, 288)[0:64]
                            rhs = q3[0:64, j0s[0] + ST + dc:j0s[0] + ST + dc + ns[0]]
                        else:
                            lhs = wsl(wlS2, t, m0, 128, 288)[64:128]
                            rhs = q3[64:128, j0s[1] + dc:j0s[1] + dc + ns[1]]
                        nc.tensor.matmul(ph[i][:, 0:ns[i]], lhs, rhs,
                                         start=False, stop=(t == 2))
                for i in range(np_):
                    e = fu.tile([128, 3 * ST], BF16, tag=f"e{ck}{i}")
                    nc.scalar.activation(e[:, 0:ns[i]], ph[i][:, 0:ns[i]],
                                         ACTF.Exp, bias=blp[:, ck:ck + 1])
                    edst.append(e)
            # chunk 2 (M=32, both strips col-packed into one [64,.] psum);
            # borrows the pm pool (idle in the tail phase) for bufs=2
            phc = pm.tile([128, 3 * ST], F32, tag="pm")
            for t, dc in enumerate((-1, 0, 1)):
                for i in range(np_):
                    o = j0s[i] - ST + dc
                    nc.tensor.matmul(phc[32 * i:32 * i + 32, 0:ns[i]],
                                     wsl(wlP, t, 256, 32, 288),
                                     q3[0:128, o:o + ns[i]],
                                     start=(t == 0), stop=False)
            for t, dc in enumerate((-1, 0, 1)):
                for i in range(np_):
                    o = j0s[i] + ST + dc
                    nc.tensor.matmul(phc[32 * i:32 * i + 32, 0:ns[i]],
                                     wsl(wlS2, t, 256, 32, 288)[0:64],
                                     q3[0:64, o:o + ns[i]],
                                     start=False, stop=(t == 2))
            for i in range(np_):
                e = fu.tile([32, 3 * ST], BF16, tag=f"ec{i}")
                nc.scalar.activation(e[:, 0:ns[i]],
                                     phc[32 * i:32 * i + 32, 0:ns[i]],
                                     ACTF.Exp, bias=blp[32 * i:32 * i + 32, 3:4])
                ec.append(e)
            # eta * patch products (DVE, same-base operands)
            ta, tb = [], []
            for i in range(np_):
                t1 = fu.tile([128, 3 * ST], BF16, tag=f"ta{i}")
                t2 = fu.tile([128, 3 * ST], BF16, tag=f"tb{i}")
                nc.vector.tensor_mul(t1[:, 0:ns[i]], ea[i][:, 0:ns[i]],
                                     msa[:, j0s[i]:j0s[i] + ns[i]])
                nc.vector.tensor_mul(t2[:, 0:ns[i]], eb[i][:, 0:ns[i]],
                                     msb[:, j0s[i]:j0s[i] + ns[i]])
                nc.vector.tensor_mul(rz[i][0:32, 0:ns[i]], ec[i][:, 0:ns[i]],
                                     ms8[:, j0s[i]:j0s[i] + ns[i]])
                ta.append(t1)
                tb.append(t2)
            return (pr, j0s, ns, rz, ea, eb, ec, ta, tb)

        def tail_fusion(state):
            pr, j0s, ns, rz, ea, eb, ec, ta, tb = state
            np_ = len(pr)
            # packed reduction matmuls: num strip i -> nd[32i:32i+32],
            # den strip i -> nd[64+32i : 96+32i]
            nd = pD.tile([128, 3 * ST], F32, tag="pD")
            for t in range(3):
                for i in range(np_):
                    npos = 32 * i
                    dpos = 64 + 32 * i
                    nl, nr_ = ((eye[:], ta[i]), (eye[:], tb[i]),
                               (eye[0:64], rz[i]))[t]
                    dl, dr = ((eye[:], ea[i]), (eye[:], eb[i]),
                              (eye[0:32], ec[i]))[t]
                    nc.tensor.matmul(nd[npos:npos + 32, 0:ns[i]], nl,
                                     nr_[:, 0:ns[i]], start=(t == 0),
                                     stop=(t == 2), tile_position=(0, npos))
                    nc.tensor.matmul(nd[dpos:dpos + 32, 0:ns[i]], dl,
                                     dr[:, 0:ns[i]], start=(t == 0),
                                     stop=(t == 2), tile_position=(0, dpos))
            # rde = 1/(den+1); ost = num * rde  (both strips at once)
            w = 32 * np_
            den = f1.tile([64, 3 * ST], F32, tag="den")
            rde = f1.tile([64, 3 * ST], F32, tag="rde")
            ost = f1.tile([64, 3 * ST], F32, tag="ost")
            nmax = max(ns)
            nc.vector.tensor_scalar_add(den[0:w, 0:nmax],
                                        nd[64:64 + w, 0:nmax], 1.0)
            nc.vector.reciprocal_approx_fast(rde[0:w, 0:nmax], den[0:w, 0:nmax])
            nc.vector.tensor_mul(ost[0:w, 0:nmax], nd[0:w, 0:nmax],
                                 rde[0:w, 0:nmax])
            for i, (r0, nr) in enumerate(pr):
                src = ost[32 * i:32 * i + 32, 0:ns[i]].rearrange(
                    "p (r c) -> p r c", c=ST)[:, :, 0:128]
                nc.sync.dma_start(out=d_out.ap()[:, r0:r0 + nr, :], in_=src)

        prev = None
        for pr in pairs:
            st = tail_compute(pr)
            if prev is not None:
                tail_fusion(prev)
            prev = st
        tail_fusion(prev)

    nc.compile()
    return nc


BF16_NP = mybir.dt.np(mybir.dt.bfloat16)


def _pad_rows(x, cols):
    # x: [C, 128, 128] -> zero-padded flat rows [C, cols], bf16
    c = x.shape[0]
    buf = np.zeros((c, cols), dtype=BF16_NP)
    buf[:, 130:130 + ST * 128].reshape(c, 128, ST)[:, :, 0:128] = x.astype(BF16_NP)
    return buf


def _shift_stack(flat, offs):
    # flat: [32, NCOL]; returns [32*len(offs), NCOL] rows shifted by offs
    ext = np.zeros((flat.shape[0], NCOL + 264), dtype=flat.dtype)
    ext[:, 132:132 + NCOL] = flat
    return np.concatenate([ext[:, 132 + o:132 + o + NCOL] for o in offs], axis=0)


def _prep_shared(w0, b0, w1, b1, w2, b2, w_last, b_last):
    f = np.float32
    w0t = np.transpose(np.asarray(w0, f), (1, 2, 3, 0))      # [160,3,3,64]
    w0c1 = np.ascontiguousarray(w0t[0:128].reshape(128, 9 * 64))
    w0c2 = np.ascontiguousarray(
        np.transpose(w0t[128:160], (1, 0, 2, 3)).reshape(96, 3 * 64))

    def mid(w):
        wt = np.transpose(np.asarray(w, f), (1, 2, 3, 0))    # [64,3,3,64]
        wP = np.ascontiguousarray(
            np.concatenate([wt[:, 0], wt[:, 1]], 0).reshape(128, 3 * 64))
        wS = np.ascontiguousarray(wt[:, 2].reshape(64, 3 * 64))
        return wP, wS

    w1P, w1S = mid(w1)
    w2P, w2S = mid(w2)
    perm = np.array([(pp % 32) * 9 + pp // 32 for pp in range(288)])
    wl2 = np.asarray(w_last, f)[perm]                        # [288,64,3,3]
    wlt = np.transpose(wl2, (1, 2, 3, 0))                    # [64,3,3,288]
    wlP = np.ascontiguousarray(
        np.concatenate([wlt[:, 0], wlt[:, 1]], 0).reshape(128, 3 * 288))
    wlS = np.ascontiguousarray(wlt[:, 2].reshape(64, 3 * 288))
    wlS2 = np.concatenate([wlS, wlS], axis=0)                # [128, 864]
    bias = np.stack([np.asarray(b0, f), np.asarray(b1, f),
                     np.asarray(b2, f)], axis=1)             # [64, 3]
    bias = np.tile(bias, (2, 1))                             # [128, 3] dup
    blf = np.asarray(b_last, f)[perm]
    blp = np.zeros((128, 4), f)
    blp[:, 0] = blf[0:128]
    blp[:, 1] = blf[128:256]
    blp[0:64, 3] = np.tile(blf[256:288], 2)
    eye = np.tile(np.eye(32, dtype=f), (4, 1))
    out = dict(w0c1=w0c1, w0c2=w0c2, w1P=w1P, w1S=w1S, w2P=w2P, w2S=w2S,
               wlP=wlP, wlS2=wlS2, eye=eye)
    out = {k: v.astype(BF16_NP) for k, v in out.items()}
    out["bias"] = np.ascontiguousarray(bias)
    out["blp"] = blp
    return out


def make_in_maps(z, backbone, mem_stab, mem_unstab, shared):
    f = np.float32
    z = np.asarray(z, f)
    backbone = np.asarray(backbone, f)
    ms = np.asarray(mem_stab, f)
    mu = np.asarray(mem_unstab, f)
    maps = []
    for b in range(z.shape[0]):
        x160 = np.concatenate([backbone[b], z[b], ms[b]], axis=0)
        msf = _pad_rows(ms[b], NCOL)
        muf = _pad_rows(mu[b], MUCOL)
        mu3 = np.concatenate([muf[:, ST * k:ST * k + MUCOL - 2 * ST - 2]
                              for k in range(3)], axis=0)
        mu3 = np.ascontiguousarray(
            np.pad(mu3, ((0, 0), (0, MUCOL - mu3.shape[1]))))
        maps.append(dict(xpad=_pad_rows(x160, NCOL),
                         mu3=mu3,
                         msa=_shift_stack(msf, P_TAPS[0:4]),
                         msb=_shift_stack(msf, P_TAPS[4:8]),
                         ms8=_shift_stack(msf, P_TAPS[8:9]),
                         **shared))
    return maps


_NC_CACHE = {}


def _get_nc(debug=False):
    if debug not in _NC_CACHE:
        _NC_CACHE[debug] = _build_program(debug)
    return _NC_CACHE[debug]


def kernel(z, backbone, mem_stab, mem_unstab, w0, b0, w1, b1, w2, b2,
           w_last, b_last, fusion_kernel_size):
    assert int(fusion_kernel_size) == 3
    shared = _prep_shared(w0, b0, w1, b1, w2, b2, w_last, b_last)
    in_maps = make_in_maps(z, backbone, mem_stab, mem_unstab, shared)
    nc = _get_nc()
    res = run_bass_kernel_spmd(nc, in_maps, core_ids=list(range(len(in_maps))))
    out = np.stack([r["out"] for r in res.results], axis=0)
    return out.astype(np.float32)
